# revision 1
# baseline (speedup 1.0000x reference)
"""Multi-head causal attention (B=2, T=2048, C=1024, H=16, D=64) on 8 trn2 cores.

Sharding: core c -> batch b = c//4, head group g = c%4 (4 heads each),
Megatron-style: QKV column-parallel, proj row-parallel. Partial outputs are
summed on the host; bk is softmax-invariant and dropped, bv/bp fold into a
host-side constant. All matmul operands are bf16 (host-cast), accumulation
and softmax stay fp32.

Device kernel (per core):
  A = x[b].T                       [1024, 2048]  host-transposed + repacked
  Q^T (+bq) / K^T = W.T @ A        [256, 2048]   channels on partitions
  V = A.T @ Wv_loc.T               [2048, 4*(64+1)]  natural layout, a ones
                                   column per head for softmax denominators
  per 512-wide q chunk qj, head h, 128-key chunk kc (diagonal chunks first,
  S emitted 3 ahead of PV so the in-order PE queue never waits on exp):
     S^T[k,q] = K_h^T.T @ Q_h^T    PSUM, trimmed to the causal q-suffix
     P^T = exp(0.125*S^T)          ACT runs exclusively Exp (no table swaps)
     diagonal chunks masked on GpSimd via affine_select
     PV~[65,*] += V~_h[kc].T @ P^T     row 64 accumulates the denominator l
     out^T = PV[0:64] * bcast(1/l)     approx-recip + gpsimd partition_broadcast
  Y = attn-out^T.T @ Wp_loc.T      [2048, 1024]  partial, proj interleaved
                                   per q-chunk, summed on host
"""

import sys

sys.path.insert(0, "/opt/trn_rl_repo")

import numpy as np
import ml_dtypes

NP_DT = ml_dtypes.bfloat16

import concourse.bass as bass  # noqa: F401
import concourse.mybir as mybir
import concourse.tile as tile
from concourse import bacc
from concourse.bass_utils import run_bass_kernel_spmd

N_CORES = 8
B, T, C = 2, 2048, 1024
H, D = 16, 64
H_LOC = 4              # heads per core
OL = H_LOC * D         # local channels = 256
CQ = 512               # PSUM-bank q chunk
CW = 1024              # exp window (2 PSUM banks)
CK = 128               # k chunk (partition dim)
NW = T // CW           # 2
NT = T // 128          # 16
KC = C // 128          # 8 contraction chunks for QKV

f32 = mybir.dt.float32
f32r = mybir.dt.float32r
bf16 = mybir.dt.bfloat16
DT = bf16  # matmul operand dtype

_COMPILED = None


def _build():
    nc = bacc.Bacc("TRN2", debug=False, num_devices=N_CORES)

    A = nc.dram_tensor("A", [2 * C, CW], DT, kind="ExternalInput").ap()
    Wqkv = nc.dram_tensor("Wqkv", [C, 3 * OL], DT, kind="ExternalInput").ap()
    WpT = nc.dram_tensor("WpT", [OL, C], DT, kind="ExternalInput").ap()
    BQ = nc.dram_tensor("BQ", [OL, 1], f32, kind="ExternalInput").ap()
    Y = nc.dram_tensor("Y", [T, C], f32, kind="ExternalOutput").ap()

    Exp = mybir.ActivationFunctionType.Exp

    with tile.TileContext(nc) as tc:
        with tc.tile_pool(name="sbuf", bufs=1) as pool, \
             tc.tile_pool(name="work", bufs=1) as wpool, \
             tc.tile_pool(name="psum", bufs=1, space="PSUM") as psum:

            # ---- resident inputs (piece-contiguous loads, weights first) ----
            a_t, w_t = [], []
            for kc in range(KC):
                at = pool.tile([128, T], DT, tag=f"A{kc}", name=f"a{kc}")
                a_t.append(at)
                wt = pool.tile([128, 3 * OL], DT, tag=f"W{kc}", name=f"w{kc}")
                w_t.append(wt)
            # A is host-repacked so block (kc, piece) = rows
            # (kc*4+piece)*128..+128 is one contiguous 128KB read. Load
            # w[kc] + piece-0 of a[kc] interleaved so the first QKV
            # accumulation chain (needs all kc) completes earliest.
            for kc in range(KC):
                nc.sync.dma_start(w_t[kc][:], Wqkv[kc * 128:(kc + 1) * 128, :])
                blk = kc * 2 * 128
                nc.sync.dma_start(
                    a_t[kc][:, 0:CW],
                    A[blk:blk + 128, 0:CW])
            for kc in range(KC):
                blk = (kc * 2 + 1) * 128
                nc.sync.dma_start(
                    a_t[kc][:, CW:T],
                    A[blk:blk + 128, 0:CW])
            wp_t = []
            for kc in range(2):
                wp = pool.tile([128, C], DT, tag=f"WP{kc}", name=f"wp{kc}")
                nc.sync.dma_start(wp[:], WpT[kc * 128:(kc + 1) * 128, :])
                wp_t.append(wp)
            bq_t = []
            for m in range(2):
                bq = pool.tile([128, 1], f32, tag=f"BQ{m}", name=f"bq{m}")
                nc.sync.dma_start(bq[:], BQ[m * 128:(m + 1) * 128, :])
                bq_t.append(bq)
            col1 = pool.tile([128, 1], f32, tag="col1")
            nc.vector.memset(col1[:], 1.0)
            # warm the GpSimd ucode paths so the first real mask/broadcast
            # doesn't eat the cold-start cost mid-attention
            warm = wpool.tile([128, 8], f32, tag="warm")
            nc.vector.memset(warm[:], 1.0)
            nc.gpsimd.affine_select(
                out=warm[:], in_=warm[:],
                compare_op=mybir.AluOpType.is_ge, fill=0.0, base=0,
                pattern=[[1, 8]], channel_multiplier=-1)
            warm2 = wpool.tile([128, 8], f32, tag="warm2")
            nc.gpsimd.partition_broadcast(warm2[:], warm[0:1, :])

            # ---- persistent intermediates ----
            qt_sb = [pool.tile([128, T], DT, tag=f"QT{i}", name=f"qt{i}")
                     for i in range(2)]
            kt_sb = [pool.tile([128, T], DT, tag=f"KT{i}", name=f"kt{i}")
                     for i in range(2)]
            v_sb = [pool.tile([128, H_LOC * (D + 1)], DT, tag=f"V{i}",
                              name=f"v{i}") for i in range(NT)]
            ao_sb = [pool.tile([128, T], DT, tag=f"AO{i}", name=f"ao{i}")
                     for i in range(2)]

            # ---- phase 1a: Q^T, K^T (evict on DVE; ACT is Exp-only) ----
            for m in range(4):
                for n in range(T // CQ):
                    ps = psum.tile([128, CQ], f32, tag="prj", bufs=2, name="ps")
                    for kc in range(KC):
                        nc.tensor.matmul(
                            ps[:],
                            w_t[kc][:, m * 128:(m + 1) * 128],
                            a_t[kc][:, n * CQ:(n + 1) * CQ],
                            start=(kc == 0), stop=(kc == KC - 1))
                    if m < 2:
                        nc.vector.tensor_scalar_add(
                            qt_sb[m][:, n * CQ:(n + 1) * CQ], ps[:],
                            bq_t[m][:, 0:1])
                    else:
                        nc.vector.tensor_copy(
                            kt_sb[m - 2][:, n * CQ:(n + 1) * CQ], ps[:])

            # ---- phase 1b: V natural layout ----
            for tt in range(NT):
                ps = psum.tile([128, CQ], f32, tag="mm", bufs=4, name="psv")[:, 0:OL]
                for kc in range(KC):
                    nc.tensor.matmul(
                        ps[:],
                        a_t[kc][:, tt * 128:(tt + 1) * 128],
                        w_t[kc][:, 2 * OL:3 * OL],
                        start=(kc == 0), stop=(kc == KC - 1))
                # single strided copy for all 4 heads' V columns frees
                # the PSUM slot ~3x sooner than 4 serial copies
                nc.vector.tensor_copy(
                    v_sb[tt].rearrange("p (h x) -> p h x", x=D + 1)[:, :, 0:D],
                    ps.rearrange("p (h x) -> p h x", x=D))
                for h in range(H_LOC):
                    nc.vector.tensor_copy(
                        v_sb[tt][:, h * (D + 1) + D:(h + 1) * (D + 1)],
                        col1[:])

            # ---- phase 2+3: causal flash attention, proj interleaved ----
            # One GLOBAL software pipeline over all (qj, h) blocks: S/exp
            # emission runs LOOKAHEAD work-items ahead of the PV consumer
            # across block boundaries, so the in-order PE queue never
            # drains/refills between blocks. Diagonal chunks first per
            # block so the GpSimd mask latency hides behind other S work.
            blocks = []
            for qj in range(T // CQ):
                for h in range(H_LOC):
                    n_kc = (qj + 1) * (CQ // CK)
                    order = list(range(qj * 4, n_kc)) + list(range(0, qj * 4))
                    blocks.append((qj, h, order))
            flat = [(bi, j) for bi, (_, _, order) in enumerate(blocks)
                    for j in range(len(order))]
            LOOKAHEAD = 3
            pv_tiles = {}
            pts = {}

            def emit_s(idx):
                bi, j = flat[idx]
                qj, h, order = blocks[bi]
                kc = order[j]
                ht, hp, q0 = h // 2, (h % 2) * 64, qj * CQ
                # diagonal chunks only cover q >= kc*CK: trim the
                # S/exp/PV stream to the causally valid q suffix
                qoff = max(0, kc * CK - q0)   # 0/128/256/384
                width = CQ - qoff
                sp = psum.tile([128, CQ], f32, tag="mm", bufs=4)
                nc.tensor.matmul(
                    sp[:, 0:width],
                    kt_sb[ht][hp:hp + D, kc * CK:(kc + 1) * CK],
                    qt_sb[ht][hp:hp + D, q0 + qoff:q0 + CQ],
                    start=True, stop=True)
                pt = wpool.tile([128, CQ], DT, tag="pT", bufs=12)
                nc.scalar.activation(pt[:, 0:width], sp[:, 0:width],
                                     Exp, scale=1.0 / 8.0)
                if kc >= qj * 4:   # diagonal chunk: mask q < k
                    nc.gpsimd.affine_select(
                        out=pt[:, 0:width], in_=pt[:, 0:width],
                        compare_op=mybir.AluOpType.is_ge,
                        fill=0.0, base=0,
                        pattern=[[1, width]], channel_multiplier=-1)
                pts[(bi, kc)] = (pt, qoff, width)

            for idx in range(min(LOOKAHEAD, len(flat))):
                emit_s(idx)
            for i, (bi, j) in enumerate(flat):
                if i + LOOKAHEAD < len(flat):
                    emit_s(i + LOOKAHEAD)
                qj, h, order = blocks[bi]
                kc = order[j]
                n_kc = len(order)
                ht, hp, q0 = h // 2, (h % 2) * 64, qj * CQ
                if j == 0:
                    pv_tiles[bi] = psum.tile([D + 1, CQ], f32, tag="pv",
                                             bufs=2, name="pv")
                pv = pv_tiles[bi]
                pt, qoff, width = pts.pop((bi, kc))
                nc.tensor.matmul(
                    pv[:, qoff:qoff + width],
                    v_sb[kc][:, h * (D + 1):(h + 1) * (D + 1)],
                    pt[:, 0:width],
                    start=(j == 0), stop=(j == n_kc - 1))
                if j != n_kc - 1:
                    continue
                # block complete: evacuate PV to SBUF (frees the PSUM slot
                # early), approx-recip the ones-row, broadcast, multiply.
                del pv_tiles[bi]
                pvs = wpool.tile([D, CQ], f32, tag="pvs", bufs=4)
                nc.vector.tensor_copy(pvs[:], pv[0:D, :])
                ls = wpool.tile([1, CQ], f32, tag="ls", bufs=2)
                nc.vector.tensor_copy(ls[:], pv[D:D + 1, :])
                r = wpool.tile([1, CQ], f32, tag="r", bufs=2)
                with nc.allow_low_precision(reason="softmax denom"):
                    # approx_fast needs SBUF input at partition base 0
                    nc.vector.reciprocal_approx_fast(r[:], ls[:])
                rbs = wpool.tile([D, CQ], f32, tag="rbs", bufs=2)
                nc.gpsimd.partition_broadcast(rbs[:], r[:])
                nc.vector.tensor_mul(
                    ao_sb[ht][hp:hp + D, q0:q0 + CQ],
                    pvs[:], rbs[:])
                if h != H_LOC - 1:
                    continue
                # all heads of this q chunk done: proj + store its token
                # tiles (overlaps the next chunk's attention)
                for tt in range(qj * (CQ // 128), (qj + 1) * (CQ // 128)):
                    for n in range(2):
                        ps = psum.tile([128, CQ], f32, tag="prj", bufs=2,
                                       name="psp")
                        for kc2 in range(2):
                            nc.tensor.matmul(
                                ps[:],
                                ao_sb[kc2][:, tt * 128:(tt + 1) * 128],
                                wp_t[kc2][:, n * CQ:(n + 1) * CQ],
                                start=(kc2 == 0), stop=(kc2 == 1))
                        yt = wpool.tile([128, CQ], f32, tag="y", bufs=4)
                        nc.vector.tensor_copy(yt[:], ps[:])
                        nc.sync.dma_start(
                            Y[tt * 128:(tt + 1) * 128, n * CQ:(n + 1) * CQ],
                            yt[:])

    nc.compile()
    return nc


def _get_compiled():
    global _COMPILED
    if _COMPILED is None:
        _COMPILED = _build()
    return _COMPILED


def make_in_maps(x, Wq, bq, Wk, Wv, Wp):
    in_maps = []
    for c in range(N_CORES):
        b, g = divmod(c, 4)
        sl = slice(g * OL, (g + 1) * OL)
        in_maps.append({
            "A": np.ascontiguousarray(
                x[b].T.reshape(KC, 128, 2, CW).transpose(0, 2, 1, 3)
                .reshape(2 * C, CW)).astype(NP_DT),
            "Wqkv": np.concatenate(
                [Wq[sl].T, Wk[sl].T, Wv[sl].T], axis=1).astype(NP_DT),
            "WpT": np.ascontiguousarray(Wp[:, sl].T).astype(NP_DT),
            "BQ": bq[sl].reshape(OL, 1).astype(np.float32),
        })
    return in_maps


_RUNNER = None


def _make_runner():
    """Build the 8-core shard_map executable once (run_bass_via_pjrt re-jits
    on every call; this caches the traced/compiled callable)."""
    import jax
    from jax.sharding import Mesh, PartitionSpec
    from jax.experimental.shard_map import shard_map
    import concourse.mybir as mybir_
    from concourse import bass2jax

    nc = _get_compiled()
    bass2jax.install_neuronx_cc_hook()

    partition_name = (nc.partition_id_tensor.name
                      if nc.partition_id_tensor else None)
    in_names, out_names, out_avals, zero_outs = [], [], [], []
    for alloc in nc.m.functions[0].allocations:
        if not isinstance(alloc, mybir_.MemoryLocationSet):
            continue
        name = alloc.memorylocations[0].name
        if alloc.kind == "ExternalInput":
            if name != partition_name:
                in_names.append(name)
        elif alloc.kind == "ExternalOutput":
            shape = tuple(alloc.tensor_shape)
            dtype = mybir_.dt.np(alloc.dtype)
            out_names.append(name)
            out_avals.append(jax.core.ShapedArray(shape, dtype))
            zero_outs.append(np.zeros(shape, dtype))
    n_params = len(in_names)
    n_outs = len(out_avals)
    all_in_names = list(in_names) + list(out_names)
    if partition_name is not None:
        all_in_names.append(partition_name)
    donate = tuple(range(n_params, n_params + n_outs))

    def _body(*args):
        operands = list(args)
        if partition_name is not None:
            operands.append(bass2jax.partition_id_tensor())
        outs = bass2jax._bass_exec_p.bind(
            *operands,
            out_avals=tuple(out_avals),
            in_names=tuple(all_in_names),
            out_names=tuple(out_names),
            lowering_input_output_aliases=(),
            sim_require_finite=True,
            sim_require_nnan=True,
            nc=nc,
        )
        return tuple(outs)

    devices = jax.devices()[:N_CORES]
    mesh = Mesh(np.asarray(devices), ("core",))
    in_specs = (PartitionSpec("core"),) * (n_params + n_outs)
    out_specs = (PartitionSpec("core"),) * n_outs
    sharded = jax.jit(
        shard_map(_body, mesh=mesh, in_specs=in_specs, out_specs=out_specs,
                  check_rep=False),
        donate_argnums=donate, keep_unused=True)

    def run(in_maps):
        per_core = [[np.asarray(m[name]) for name in in_names]
                    for m in in_maps]
        concat_in = [
            np.concatenate([per_core[c][i] for c in range(N_CORES)], axis=0)
            for i in range(n_params)]
        concat_zeros = [
            np.zeros((N_CORES * z.shape[0], *z.shape[1:]), z.dtype)
            for z in zero_outs]
        out_arrs = sharded(*concat_in, *concat_zeros)
        return [
            {name: np.asarray(out_arrs[i]).reshape(
                N_CORES, *out_avals[i].shape)[c]
             for i, name in enumerate(out_names)}
            for c in range(N_CORES)]

    return run


def _get_runner():
    global _RUNNER
    if _RUNNER is None:
        _RUNNER = _make_runner()
    return _RUNNER


def _axon_reset():
    try:
        import ctypes
        lib = ctypes.CDLL("/opt/axon/libaxon_pjrt.so")
        if hasattr(lib, "axon_reset"):
            lib.axon_reset()
    except Exception:
        pass


def kernel(x, Wq, bq, Wk, bk, Wv, bv, Wp, bp):
    x = np.asarray(x, dtype=np.float32)
    Wq = np.asarray(Wq, dtype=np.float32)
    bq = np.asarray(bq, dtype=np.float32)
    Wk = np.asarray(Wk, dtype=np.float32)
    Wv = np.asarray(Wv, dtype=np.float32)
    Wp = np.asarray(Wp, dtype=np.float32)
    bv = np.asarray(bv, dtype=np.float32)
    bp = np.asarray(bp, dtype=np.float32)

    in_maps = make_in_maps(x, Wq, bq, Wk, Wv, Wp)

    results = None
    for attempt in range(3):
        try:
            results = _get_runner()(in_maps)
            break
        except Exception:
            if attempt == 2:
                raise
            _axon_reset()  # recover a wedged accelerator and retry

    extra = bv @ Wp.T + bp  # bv/bp fold out of the device kernel
    out = np.empty((B, T, C), dtype=np.float32)
    for b in range(B):
        acc = results[4 * b]["Y"].astype(np.float32)
        for g in range(1, 4):
            acc = acc + results[4 * b + g]["Y"]
        out[b] = acc + extra
    return out



# revision 51
# speedup vs baseline: 1.2881x; 1.2881x over previous
"""Multi-head causal attention (B=2, T=2048, C=1024, H=16, D=64) on 8 trn2 cores.

Sharding: core c -> batch b = c//4, head group g = c%4 (4 heads each),
Megatron-style: QKV column-parallel, proj row-parallel. Partial outputs are
summed on the host; bk is softmax-invariant and dropped, bv/bp fold into a
host-side constant. All matmul operands are bf16 (host-cast), accumulation
and softmax stay fp32.

v2: the attention window is paced by the Scalar engine (exp is Scalar-only at
~0.83ns/row), so the kernel keeps Scalar 100% on Exp and moves everything
else off it:
  - exp runs on [128, <=1024] paired S tiles (2 PSUM banks) - 80 calls, no
    wasted rows (diag chunks pack densely).
  - causal mask = PE matmul accumulating a -1e9 triangle constant into S
    BEFORE exp (no GpSimd affine_select, no post-exp dependency).
  - softmax denominators: ones-column in V gives l in PV row 64; GpSimd
    copies it out, DVE approx-reciprocal, PE broadcasts r back into PV rows
    64:128 via an f32r rank-1 matmul, one DVE multiply -> ao.
  - QKV phase 1 runs kc-outer over 8 concurrent PSUM chains so the PE starts
    ~1us in (first weight chunk) instead of waiting for the full load; only
    the first half (q cols 0:1024) runs up front - the second half, V[tt>=8]
    and the per-qj projection are interleaved as background PE work under
    the Scalar-bound attention window.
  - Y is stored bf16 (host sums partials in f32).
"""

import sys

sys.path.insert(0, "/opt/trn_rl_repo")

import numpy as np
import ml_dtypes

NP_DT = ml_dtypes.bfloat16

import concourse.bass as bass  # noqa: F401
import concourse.mybir as mybir
import concourse.tile as tile
from concourse import bacc
from concourse.bass_utils import run_bass_kernel_spmd  # noqa: F401

N_CORES = 8
B, T, C = 2, 2048, 1024
H, D = 16, 64
H_LOC = 4              # heads per core
OL = H_LOC * D         # local channels = 256
CQ = 512               # q chunk (PSUM bank / block granularity)
CK = 128               # k chunk (partition dim)
NT = T // 128          # 16
KC = C // 128          # 8 contraction chunks for QKV
VW = D + 1             # 65: V columns per head incl ones column

f32 = mybir.dt.float32
f32r = mybir.dt.float32r
bf16 = mybir.dt.bfloat16
DT = bf16

USE_PE_MASK = False    # mask via PE matmul add of -1e9 triangle (else gpsimd)
USE_F32R_BCAST = False  # broadcast 1/l via f32r PE matmul (else gpsimd)
DEBUG_DUMP = False     # add QT/KT/V/AO dram dumps for debugging

_COMPILED = None


def _build():
    nc = bacc.Bacc("TRN2", debug=False, num_devices=N_CORES)

    A = nc.dram_tensor("A", [2 * C, T // 2], DT, kind="ExternalInput").ap()
    Wqkv = nc.dram_tensor("Wqkv", [C, 3 * OL], DT, kind="ExternalInput").ap()
    WpT = nc.dram_tensor("WpT", [OL, C], DT, kind="ExternalInput").ap()
    BQ = nc.dram_tensor("BQ", [OL, 1], f32, kind="ExternalInput").ap()
    CST = nc.dram_tensor("CST", [128, 256], DT, kind="ExternalInput").ap()
    Y = nc.dram_tensor("Y", [T, C], DT, kind="ExternalOutput").ap()
    if DEBUG_DUMP:
        DQT = nc.dram_tensor("DQT", [256, T], DT, kind="ExternalOutput").ap()
        DKT = nc.dram_tensor("DKT", [256, T], DT, kind="ExternalOutput").ap()
        DV = nc.dram_tensor("DV", [128, NT * H_LOC * VW], DT,
                            kind="ExternalOutput").ap()
        DAO = nc.dram_tensor("DAO", [256, T], DT, kind="ExternalOutput").ap()

    Exp = mybir.ActivationFunctionType.Exp
    Ident = mybir.ActivationFunctionType.Identity

    with tile.TileContext(nc) as tc:
        with tc.tile_pool(name="sbuf", bufs=1) as pool, \
             tc.tile_pool(name="work", bufs=1) as wpool, \
             tc.tile_pool(name="psum", bufs=1, space="PSUM") as psum:

            # ---------------- resident inputs ----------------
            a_t = [pool.tile([128, T], DT, tag=f"A{kc}", name=f"a{kc}")
                   for kc in range(KC)]
            w_t = [pool.tile([128, 3 * OL], DT, tag=f"W{kc}", name=f"w{kc}")
                   for kc in range(KC)]
            # kc=0 pair first, in strips (parallel DMA engines finish each
            # strip sooner), so the first QKV chain starts as early as
            # possible; bq/cst are tiny and only needed ~10us in.
            bq_t = [pool.tile([128, 1], f32, tag=f"BQ{m}", name=f"bq{m}")
                    for m in range(2)]
            cst = pool.tile([128, 256], DT, tag="CST", name="cst")
            for s in range(2):
                nc.sync.dma_start(w_t[0][:, s * 384:(s + 1) * 384],
                                  Wqkv[0:128, s * 384:(s + 1) * 384])
            for s in range(2):
                nc.sync.dma_start(a_t[0][:, s * CQ:(s + 1) * CQ],
                                  A[0:128, s * CQ:(s + 1) * CQ])
            # block (kc, piece) = rows (kc*2+piece)*128..+128 of A is one
            # contiguous read; weights + piece0 interleaved so the kc-outer
            # QKV chains can start after the first pair lands.
            for kc in range(1, KC):
                nc.sync.dma_start(w_t[kc][:], Wqkv[kc * 128:(kc + 1) * 128, :])
                blk = kc * 2 * 128
                nc.sync.dma_start(a_t[kc][:, 0:T // 2],
                                  A[blk:blk + 128, :])
            for m in range(2):
                nc.sync.dma_start(bq_t[m][:], BQ[m * 128:(m + 1) * 128, :])
            nc.sync.dma_start(cst[:], CST[:, :])
            for kc in range(KC):
                blk = (kc * 2 + 1) * 128
                nc.sync.dma_start(a_t[kc][:, T // 2:T],
                                  A[blk:blk + 128, :])
            wp_t = []
            for kc in range(2):
                wp = pool.tile([128, C], DT, tag=f"WP{kc}", name=f"wp{kc}")
                nc.sync.dma_start(wp[:], WpT[kc * 128:(kc + 1) * 128, :])
                wp_t.append(wp)
            ident_t = cst[:, 0:128]    # identity
            tri_t = cst[:, 128:256]    # -1e9 strictly-lower triangle (r > j)

            ones_r = pool.tile([1, 64], f32, tag="ones1")
            nc.vector.memset(ones_r[:], 1.0)

            # persistent intermediates
            qt_sb = [pool.tile([128, T], DT, tag=f"QT{i}", name=f"qt{i}")
                     for i in range(2)]
            # per-head K aligned with where Q_h sits in the stacked QT tile
            # (rows hp:hp+64), other 64 rows zero - S matmuls then run
            # contraction 128 (contraction<=64 executes at half PE rate)
            # and the other head's Q rows hit the zeros
            kt_z = [pool.tile([128, T], DT, tag=f"KZ{h}", name=f"ktz{h}")
                    for h in range(H_LOC)]
            for h in range(H_LOC):
                zp = 0 if h % 2 else D
                nc.vector.memset(kt_z[h][zp:zp + D, :], 0.0)
            # V natural layout, all 16 token-tiles in one tile:
            # slice (tt, h) = [:, tt*4*VW + h*VW : +VW], col 64 = ones.
            v_all = pool.tile([128, NT * H_LOC * VW], DT, tag="VALL",
                              name="v_all")
            ao_sb = [pool.tile([128, T], DT, tag=f"AO{i}", name=f"ao{i}")
                     for i in range(2)]

            # ones columns of V: one strided memset covers all (tt, h)
            nc.vector.memset(
                v_all.rearrange("p (t x) -> p t x", x=VW)[:, :, D:D + 1], 1.0)

            # warm dve custom-op ucode path
            warm = wpool.tile([1, 8], f32, tag="warm")
            nc.vector.memset(warm[:], 1.0)
            warm2 = wpool.tile([1, 8], f32, tag="warm2")
            nc.vector.tensor_copy(warm2[:], warm[:])
            with nc.allow_low_precision(reason="warmup"):
                nc.vector.reciprocal_approx_fast(warm[:], warm2[:])
            if not USE_PE_MASK:
                warm3 = wpool.tile([128, 8], f32, tag="warm3")
                nc.vector.memset(warm3[:], 1.0)
                nc.gpsimd.affine_select(
                    out=warm3[:], in_=warm3[:],
                    compare_op=mybir.AluOpType.is_ge, fill=0.0, base=0,
                    pattern=[[1, 8]], channel_multiplier=-1)
            if not USE_F32R_BCAST:
                warm4 = wpool.tile([64, 8], f32, tag="warm4")
                nc.gpsimd.partition_broadcast(warm4[:], warm2[:])

            # ---------------- phase 1a: QKV halfA (q cols 0:1024) ----------
            # kc-outer over 8 chains: m in {q0,q1,k0,k1} x n in {0,1}.
            # slots: q0/q1 -> mm bufs, k0 -> pv bufs, k1 -> prj bufs
            mmA = psum.tile([128, 2 * CQ], f32, tag="mm", bufs=2, name="mmA")
            mmB = psum.tile([128, 2 * CQ], f32, tag="mm", bufs=2, name="mmB")
            pvA = psum.tile([128, CQ], f32, tag="pv", bufs=2, name="pvA")
            pvB = psum.tile([128, CQ], f32, tag="pv", bufs=2, name="pvB")
            prA = psum.tile([128, CQ], f32, tag="prj", bufs=2, name="prA")
            prB = psum.tile([128, CQ], f32, tag="prj", bufs=2, name="prB")
            halfA_slots = {
                (0, 0): mmA[:, 0:CQ], (0, 1): mmA[:, CQ:2 * CQ],
                (1, 0): mmB[:, 0:CQ], (1, 1): mmB[:, CQ:2 * CQ],
                (2, 0): pvA[:], (2, 1): pvB[:],
                (3, 0): prA[:], (3, 1): prB[:],
            }
            for kc in range(KC):
                for m in range(4):
                    for n in range(2):
                        nc.tensor.matmul(
                            halfA_slots[(m, n)],
                            w_t[kc][:, m * 128:(m + 1) * 128],
                            a_t[kc][:, n * CQ:(n + 1) * CQ],
                            start=(kc == 0), stop=(kc == KC - 1))
            # evict: Q via Scalar (fused bias add), K via DVE split per head
            for m in range(2):
                nc.scalar.activation(
                    qt_sb[m][:, 0:2 * CQ], (mmA if m == 0 else mmB)[:],
                    Ident, bias=bq_t[m][:, 0:1], scale=1.0)
            for ps_, h2, cols in [(pvA, 0, slice(0, CQ)),
                                  (pvB, 0, slice(CQ, 2 * CQ)),
                                  (prA, 2, slice(0, CQ)),
                                  (prB, 2, slice(CQ, 2 * CQ))]:
                nc.vector.tensor_copy(kt_z[h2][0:D, cols], ps_[0:D, :])
                nc.vector.tensor_copy(kt_z[h2 + 1][D:128, cols], ps_[D:128, :])

            # ---------------- attention stream -----------------------------
            # blocks (qj, h): chunks = diagonals (kc=qj*4..qj*4+3, trimmed to
            # the causal q-suffix) then off-diagonals (kc=0..qj*4-1).
            # Chunks pack pairwise into [128, <=1024] S tiles; one exp per
            # tile. PV accumulates per block in pv psum rows 0:65 (row 64 =
            # softmax denominator via the V ones column).
            class Chunk:
                __slots__ = ("kc", "qoff", "width", "off", "diag", "idx")

            class Tile:
                __slots__ = ("chunks", "ext", "blk", "first", "last", "pt")

            class Block:
                __slots__ = ("qj", "h", "tiles", "pv", "ls", "r", "rbs",
                             "n_chunks")

            blocks = []
            all_tiles = []
            for qj in range(4):
                for h in range(H_LOC):
                    blk = Block()
                    blk.qj, blk.h = qj, h
                    chunks = []
                    for kc in range(qj * 4):    # off-diagonal chunks first
                        c = Chunk()
                        c.kc, c.qoff, c.width, c.diag = kc, 0, CQ, False
                        chunks.append(c)
                    for i in range(4):          # diagonal chunks (trimmed)
                        c = Chunk()
                        c.kc = qj * 4 + i
                        c.qoff = i * 128
                        c.width = CQ - c.qoff
                        c.diag = True
                        chunks.append(c)
                    for ci, c in enumerate(chunks):
                        c.idx = ci
                    blk.n_chunks = len(chunks)
                    # pack pairs into [128,1024] tiles; each chunk owns one
                    # PSUM bank (accumulation groups must not share a bank),
                    # so the second chunk always starts at col 512
                    tiles = []
                    for j in range(0, len(chunks), 2):
                        t = Tile()
                        pair = chunks[j:j + 2]
                        pair[0].off = 0
                        if len(pair) == 2:
                            pair[1].off = CQ
                            t.ext = CQ + pair[1].width
                        else:
                            t.ext = pair[0].width
                        t.chunks = pair
                        t.blk = blk
                        tiles.append(t)
                    for ti, t in enumerate(tiles):
                        t.first = (ti == 0)
                        t.last = (ti == len(tiles) - 1)
                    blk.tiles = tiles
                    blocks.append(blk)
                    all_tiles.extend(tiles)

            n_tiles = len(all_tiles)

            def emit_S(t):
                blk = t.blk
                ht, hp = blk.h // 2, (blk.h % 2) * 64
                q0 = blk.qj * CQ
                sp = psum.tile([128, 2 * CQ], f32, tag="mm", bufs=2,
                               name="sp")
                for c in t.chunks:
                    nc.tensor.matmul(
                        sp[:, c.off:c.off + c.width],
                        kt_z[blk.h][:, c.kc * CK:(c.kc + 1) * CK],
                        qt_sb[ht][:, q0 + c.qoff:q0 + CQ],
                        start=True, stop=not (USE_PE_MASK and c.diag))
                    if USE_PE_MASK and c.diag:
                        nc.tensor.matmul(
                            sp[:, c.off:c.off + CK],
                            ident_t, tri_t,
                            start=False, stop=True, skip_group_check=True)
                pt = wpool.tile([128, 2 * CQ], DT, tag="pT", bufs=6)
                nc.scalar.activation(pt[:, 0:t.ext], sp[:, 0:t.ext],
                                     Exp, scale=1.0 / 8.0)
                if not USE_PE_MASK:
                    for c in t.chunks:
                        if c.diag:
                            w = min(CK, c.width)
                            nc.gpsimd.affine_select(
                                out=pt[:, c.off:c.off + w],
                                in_=pt[:, c.off:c.off + w],
                                compare_op=mybir.AluOpType.is_ge,
                                fill=0.0, base=0,
                                pattern=[[1, w]], channel_multiplier=-1)
                t.pt = pt

            def emit_PV(t):
                blk = t.blk
                if t.first:
                    blk.pv = psum.tile([128, CQ], f32, tag="pv", bufs=2,
                                       name="pv")
                for c in t.chunks:
                    nc.tensor.matmul(
                        blk.pv[0:VW, c.qoff:c.qoff + c.width],
                        v_all[:, c.kc * H_LOC * VW + blk.h * VW:
                              c.kc * H_LOC * VW + (blk.h + 1) * VW],
                        t.pt[:, c.off:c.off + c.width],
                        start=(c.idx == 0), stop=(c.idx == blk.n_chunks - 1))

            def emit_ls_recip(blk):
                ls = wpool.tile([1, CQ], f32, tag="ls", bufs=2)
                nc.vector.tensor_copy(ls[:], blk.pv[D:D + 1, :])
                r = wpool.tile([1, CQ], f32, tag="r", bufs=2)
                with nc.allow_low_precision(reason="softmax denom"):
                    nc.vector.reciprocal_approx_fast(r[:], ls[:])
                blk.ls, blk.r = ls, r

            def emit_bcast(blk):
                if USE_F32R_BCAST:
                    nc.tensor.matmul(
                        blk.pv[D:D + D, :],
                        ones_r[:].bitcast(f32r),
                        blk.r[:].bitcast(f32r),
                        start=True, stop=True, skip_group_check=True)
                else:
                    rbs = wpool.tile([D, CQ], f32, tag="rbs", bufs=2)
                    nc.gpsimd.partition_broadcast(rbs[:], blk.r[:])
                    blk.rbs = rbs

            def emit_mul(blk):
                ht, hp = blk.h // 2, (blk.h % 2) * 64
                q0 = blk.qj * CQ
                rhs = blk.pv[D:2 * D, :] if USE_F32R_BCAST else blk.rbs[:]
                nc.vector.tensor_mul(
                    ao_sb[ht][hp:hp + D, q0:q0 + CQ],
                    blk.pv[0:D, :], rhs)

            # background PE work: halfB QKV, V tt8-15, proj per qj
            background = []

            def bg_halfB(m, n):
                def run():
                    ps = psum.tile([128, CQ], f32, tag="prj", bufs=2,
                                   name="hb")
                    for kc in range(KC):
                        nc.tensor.matmul(
                            ps[:], w_t[kc][:, m * 128:(m + 1) * 128],
                            a_t[kc][:, n * CQ:(n + 1) * CQ],
                            start=(kc == 0), stop=(kc == KC - 1))
                    if m < 2:
                        nc.vector.tensor_scalar_add(
                            qt_sb[m][:, n * CQ:(n + 1) * CQ], ps[:],
                            bq_t[m][:, 0:1])
                    else:
                        h2 = (m - 2) * 2
                        cols = slice(n * CQ, (n + 1) * CQ)
                        nc.vector.tensor_copy(kt_z[h2][0:D, cols],
                                              ps[0:D, :])
                        nc.vector.tensor_copy(kt_z[h2 + 1][D:128, cols],
                                              ps[D:128, :])
                return run

            def bg_v(tt):
                def run():
                    ps = psum.tile([128, CQ], f32, tag="prj", bufs=2,
                                   name="vb")
                    for kc in range(KC):
                        nc.tensor.matmul(
                            ps[:, 0:OL],
                            a_t[kc][:, tt * 128:(tt + 1) * 128],
                            w_t[kc][:, 2 * OL:3 * OL],
                            start=(kc == 0), stop=(kc == KC - 1))
                    dst = v_all.rearrange("p (t h x) -> p t h x",
                                          h=H_LOC, x=VW)[:, tt, :, 0:D]
                    src = ps[:, 0:OL].rearrange("p (h x) -> p h x", x=D)
                    nc.vector.tensor_copy(dst, src)
                return run

            def bg_proj(tt, n, last_qj):
                def run():
                    ps = psum.tile([128, CQ], f32, tag="prj", bufs=2,
                                   name="pj")
                    for kc2 in range(2):
                        nc.tensor.matmul(
                            ps[:],
                            ao_sb[kc2][:, tt * 128:(tt + 1) * 128],
                            wp_t[kc2][:, n * CQ:(n + 1) * CQ],
                            start=(kc2 == 0), stop=(kc2 == 1))
                    yt = wpool.tile([128, CQ], DT, tag="y", bufs=6)
                    if last_qj and n == 1:
                        # Scalar is idle after the final exp; split the tail
                        # evictions across engines
                        nc.scalar.copy(yt[:], ps[:])
                    else:
                        nc.vector.tensor_copy(yt[:], ps[:])
                    nc.sync.dma_start(
                        Y[tt * 128:(tt + 1) * 128, n * CQ:(n + 1) * CQ],
                        yt[:])
                return run

            # background order: V tt0-7 first (v0-3 popped before the loop,
            # the rest under qj0's exp cover), halfB n=2 (needed by qj2 S),
            # V tt8-11 (qj2 PV), halfB n=3 (qj3 S), V tt12-15 (qj3 PV);
            # proj chunks are appended as their qj completes.
            for tt in range(0, 8):
                background.append(bg_v(tt))
            for m in range(4):
                background.append(bg_halfB(m, 2))
            for tt in range(8, 12):
                background.append(bg_v(tt))
            for m in range(4):
                background.append(bg_halfB(m, 3))
            for tt in range(12, 16):
                background.append(bg_v(tt))

            # schedule: software pipeline over tiles with lookahead LA for
            # S/exp; block-end ops lag to keep engine queues stall-free.
            LA = 2
            pend_bcast = []   # (due_step, blk)
            pend_mul = []     # (due_step, blk)

            for i in range(min(LA, n_tiles)):
                emit_S(all_tiles[i])
            for _ in range(4):            # V tt0-3: needed by the first PVs
                background.pop(0)()
            quota = [4, 4, 8, 999]        # background pops allowed per qj

            for i, t in enumerate(all_tiles):
                for due, blk in [p for p in pend_bcast if p[0] <= i]:
                    emit_bcast(blk)
                    pend_bcast.remove((due, blk))
                if i + LA < n_tiles:
                    emit_S(all_tiles[i + LA])
                emit_PV(t)
                if t.last:
                    blk = t.blk
                    emit_ls_recip(blk)
                    pend_bcast.append((i + 1, blk))
                    pend_mul.append((i + 2, blk))
                for due, blk in [p for p in pend_mul if p[0] <= i]:
                    emit_mul(blk)
                    pend_mul.remove((due, blk))
                    if blk.h == H_LOC - 1:
                        qj = blk.qj
                        for tt in range(qj * 4, (qj + 1) * 4):
                            for n in range(2):
                                background.append(
                                    bg_proj(tt, n, qj == 3))
                # pop background work under the exp cover, budgeted per qj
                # so the in-order PE queue never starves Scalar of S tiles
                # (qj3 has the most Scalar slack, so proj work lands there).
                # Near the end, stretch the last few ops so the PE has fill
                # work while Scalar finishes the final exps.
                if background:
                    qj = t.blk.qj
                    backlog = len(background)
                    pop = quota[qj] > 0 or backlog >= n_tiles - i
                    if (qj == 3 and backlog <= 8 and i % 2 == 0
                            and backlog < n_tiles - i):
                        pop = False
                    if pop:
                        background.pop(0)()
                        quota[qj] -= 1

            # drain
            for _, blk in pend_bcast:
                emit_bcast(blk)
            for _, blk in pend_mul:
                emit_mul(blk)
                if blk.h == H_LOC - 1:
                    qj = blk.qj
                    for tt in range(qj * 4, (qj + 1) * 4):
                        for n in range(2):
                            background.append(bg_proj(tt, n, qj == 3))
            while background:
                background.pop(0)()

            if DEBUG_DUMP:
                for i in range(2):
                    nc.sync.dma_start(DQT[i * 128:(i + 1) * 128, :],
                                      qt_sb[i][:])
                    nc.sync.dma_start(DAO[i * 128:(i + 1) * 128, :],
                                      ao_sb[i][:])
                for h in range(H_LOC):
                    hp = (h % 2) * D
                    nc.sync.dma_start(DKT[h * D:(h + 1) * D, :],
                                      kt_z[h][hp:hp + D, :])
                nc.sync.dma_start(DV[:, :], v_all[:])

    nc.compile()
    return nc


def _get_compiled():
    global _COMPILED
    if _COMPILED is None:
        _COMPILED = _build()
    return _COMPILED


def _make_consts():
    ident = np.eye(128, dtype=np.float32)
    tri = np.zeros((128, 128), dtype=np.float32)
    r_idx = np.arange(128)[:, None]
    j_idx = np.arange(128)[None, :]
    tri[r_idx > j_idx] = -1e9
    return np.concatenate([ident, tri], axis=1).astype(NP_DT)


def make_in_maps(x, Wq, bq, Wk, Wv, Wp):
    cst = _make_consts()
    in_maps = []
    for c in range(N_CORES):
        b, g = divmod(c, 4)
        sl = slice(g * OL, (g + 1) * OL)
        in_maps.append({
            "A": np.ascontiguousarray(
                x[b].T.reshape(KC, 128, 2, T // 2).transpose(0, 2, 1, 3)
                .reshape(2 * C, T // 2)).astype(NP_DT),
            "Wqkv": np.concatenate(
                [Wq[sl].T, Wk[sl].T, Wv[sl].T], axis=1).astype(NP_DT),
            "WpT": np.ascontiguousarray(Wp[:, sl].T).astype(NP_DT),
            "BQ": bq[sl].reshape(OL, 1).astype(np.float32),
            "CST": cst,
        })
    return in_maps


_RUNNER = None


def _make_runner():
    """Build the 8-core shard_map executable once (run_bass_via_pjrt re-jits
    on every call; this caches the traced/compiled callable)."""
    import jax
    from jax.sharding import Mesh, PartitionSpec
    from jax.experimental.shard_map import shard_map
    import concourse.mybir as mybir_
    from concourse import bass2jax

    nc = _get_compiled()
    bass2jax.install_neuronx_cc_hook()

    partition_name = (nc.partition_id_tensor.name
                      if nc.partition_id_tensor else None)
    in_names, out_names, out_avals, zero_outs = [], [], [], []
    for alloc in nc.m.functions[0].allocations:
        if not isinstance(alloc, mybir_.MemoryLocationSet):
            continue
        name = alloc.memorylocations[0].name
        if alloc.kind == "ExternalInput":
            if name != partition_name:
                in_names.append(name)
        elif alloc.kind == "ExternalOutput":
            shape = tuple(alloc.tensor_shape)
            dtype = mybir_.dt.np(alloc.dtype)
            out_names.append(name)
            out_avals.append(jax.core.ShapedArray(shape, dtype))
            zero_outs.append(np.zeros(shape, dtype))
    n_params = len(in_names)
    n_outs = len(out_avals)
    all_in_names = list(in_names) + list(out_names)
    if partition_name is not None:
        all_in_names.append(partition_name)
    donate = tuple(range(n_params, n_params + n_outs))

    def _body(*args):
        operands = list(args)
        if partition_name is not None:
            operands.append(bass2jax.partition_id_tensor())
        outs = bass2jax._bass_exec_p.bind(
            *operands,
            out_avals=tuple(out_avals),
            in_names=tuple(all_in_names),
            out_names=tuple(out_names),
            lowering_input_output_aliases=(),
            sim_require_finite=True,
            sim_require_nnan=True,
            nc=nc,
        )
        return tuple(outs)

    devices = jax.devices()[:N_CORES]
    mesh = Mesh(np.asarray(devices), ("core",))
    in_specs = (PartitionSpec("core"),) * (n_params + n_outs)
    out_specs = (PartitionSpec("core"),) * n_outs
    sharded = jax.jit(
        shard_map(_body, mesh=mesh, in_specs=in_specs, out_specs=out_specs,
                  check_rep=False),
        donate_argnums=donate, keep_unused=True)

    def run(in_maps):
        per_core = [[np.asarray(m[name]) for name in in_names]
                    for m in in_maps]
        concat_in = [
            np.concatenate([per_core[c][i] for c in range(N_CORES)], axis=0)
            for i in range(n_params)]
        concat_zeros = [
            np.zeros((N_CORES * z.shape[0], *z.shape[1:]), z.dtype)
            for z in zero_outs]
        out_arrs = sharded(*concat_in, *concat_zeros)
        return [
            {name: np.asarray(out_arrs[i]).reshape(
                N_CORES, *out_avals[i].shape)[c]
             for i, name in enumerate(out_names)}
            for c in range(N_CORES)]

    return run


def _get_runner():
    global _RUNNER
    if _RUNNER is None:
        _RUNNER = _make_runner()
    return _RUNNER


def _axon_reset():
    try:
        import ctypes
        lib = ctypes.CDLL("/opt/axon/libaxon_pjrt.so")
        if hasattr(lib, "axon_reset"):
            lib.axon_reset()
    except Exception:
        pass


def kernel(x, Wq, bq, Wk, bk, Wv, bv, Wp, bp):
    x = np.asarray(x, dtype=np.float32)
    Wq = np.asarray(Wq, dtype=np.float32)
    bq = np.asarray(bq, dtype=np.float32)
    Wk = np.asarray(Wk, dtype=np.float32)
    Wv = np.asarray(Wv, dtype=np.float32)
    Wp = np.asarray(Wp, dtype=np.float32)
    bv = np.asarray(bv, dtype=np.float32)
    bp = np.asarray(bp, dtype=np.float32)

    in_maps = make_in_maps(x, Wq, bq, Wk, Wv, Wp)

    results = None
    for attempt in range(3):
        try:
            results = _get_runner()(in_maps)
            break
        except Exception:
            if attempt == 2:
                raise
            _axon_reset()  # recover a wedged accelerator and retry

    extra = bv @ Wp.T + bp  # bv/bp fold out of the device kernel
    out = np.empty((B, T, C), dtype=np.float32)
    for b in range(B):
        acc = results[4 * b]["Y"].astype(np.float32)
        for g in range(1, 4):
            acc = acc + results[4 * b + g]["Y"].astype(np.float32)
        out[b] = acc + extra
    return out


# revision 54
# speedup vs baseline: 1.2910x; 1.0022x over previous
"""Multi-head causal attention (B=2, T=2048, C=1024, H=16, D=64) on 8 trn2 cores.

Sharding: core c -> batch b = c//4, head group g = c%4 (4 heads each),
Megatron-style: QKV column-parallel, proj row-parallel. Partial outputs are
summed on the host; bk is softmax-invariant and dropped, bv/bp fold into a
host-side constant. All matmul operands are bf16 (host-cast), accumulation
and softmax stay fp32.

v2: the attention window is paced by the Scalar engine (exp is Scalar-only at
~0.83ns/row), so the kernel keeps Scalar 100% on Exp and moves everything
else off it:
  - exp runs on [128, <=1024] paired S tiles (2 PSUM banks) - 80 calls, no
    wasted rows (diag chunks pack densely).
  - causal mask = PE matmul accumulating a -1e9 triangle constant into S
    BEFORE exp (no GpSimd affine_select, no post-exp dependency).
  - softmax denominators: ones-column in V gives l in PV row 64; GpSimd
    copies it out, DVE approx-reciprocal, PE broadcasts r back into PV rows
    64:128 via an f32r rank-1 matmul, one DVE multiply -> ao.
  - QKV phase 1 runs kc-outer over 8 concurrent PSUM chains so the PE starts
    ~1us in (first weight chunk) instead of waiting for the full load; only
    the first half (q cols 0:1024) runs up front - the second half, V[tt>=8]
    and the per-qj projection are interleaved as background PE work under
    the Scalar-bound attention window.
  - Y is stored bf16 (host sums partials in f32).
"""

import sys

sys.path.insert(0, "/opt/trn_rl_repo")

import numpy as np
import ml_dtypes

NP_DT = ml_dtypes.bfloat16

import concourse.bass as bass  # noqa: F401
import concourse.mybir as mybir
import concourse.tile as tile
from concourse import bacc
from concourse.bass_utils import run_bass_kernel_spmd  # noqa: F401

N_CORES = 8
B, T, C = 2, 2048, 1024
H, D = 16, 64
H_LOC = 4              # heads per core
OL = H_LOC * D         # local channels = 256
CQ = 512               # q chunk (PSUM bank / block granularity)
CK = 128               # k chunk (partition dim)
NT = T // 128          # 16
KC = C // 128          # 8 contraction chunks for QKV
VW = D + 1             # 65: V columns per head incl ones column

f32 = mybir.dt.float32
f32r = mybir.dt.float32r
bf16 = mybir.dt.bfloat16
DT = bf16

USE_PE_MASK = False    # mask via PE matmul add of -1e9 triangle (else gpsimd)
USE_F32R_BCAST = False  # broadcast 1/l via f32r PE matmul (else gpsimd)
DEBUG_DUMP = False     # add QT/KT/V/AO dram dumps for debugging

_COMPILED = None


def _build():
    nc = bacc.Bacc("TRN2", debug=False, num_devices=N_CORES)

    A = nc.dram_tensor("A", [2 * C, T // 2], DT, kind="ExternalInput").ap()
    Wqkv = nc.dram_tensor("Wqkv", [C, 3 * OL], DT, kind="ExternalInput").ap()
    WpT = nc.dram_tensor("WpT", [OL, C], DT, kind="ExternalInput").ap()
    BQ = nc.dram_tensor("BQ", [OL, 1], f32, kind="ExternalInput").ap()
    CST = nc.dram_tensor("CST", [128, 256], DT, kind="ExternalInput").ap()
    Y = nc.dram_tensor("Y", [T, C], DT, kind="ExternalOutput").ap()
    if DEBUG_DUMP:
        DQT = nc.dram_tensor("DQT", [256, T], DT, kind="ExternalOutput").ap()
        DKT = nc.dram_tensor("DKT", [256, T], DT, kind="ExternalOutput").ap()
        DV = nc.dram_tensor("DV", [128, NT * H_LOC * VW], DT,
                            kind="ExternalOutput").ap()
        DAO = nc.dram_tensor("DAO", [256, T], DT, kind="ExternalOutput").ap()

    Exp = mybir.ActivationFunctionType.Exp
    Ident = mybir.ActivationFunctionType.Identity

    with tile.TileContext(nc) as tc:
        with tc.tile_pool(name="sbuf", bufs=1) as pool, \
             tc.tile_pool(name="work", bufs=1) as wpool, \
             tc.tile_pool(name="psum", bufs=1, space="PSUM") as psum:

            # ---------------- resident inputs ----------------
            a_t = [pool.tile([128, T], DT, tag=f"A{kc}", name=f"a{kc}")
                   for kc in range(KC)]
            w_t = [pool.tile([128, 3 * OL], DT, tag=f"W{kc}", name=f"w{kc}")
                   for kc in range(KC)]
            # kc=0 pair first, in strips (parallel DMA engines finish each
            # strip sooner), so the first QKV chain starts as early as
            # possible; bq/cst are tiny and only needed ~10us in.
            bq_t = [pool.tile([128, 1], f32, tag=f"BQ{m}", name=f"bq{m}")
                    for m in range(2)]
            cst = pool.tile([128, 256], DT, tag="CST", name="cst")
            for s in range(2):
                nc.sync.dma_start(w_t[0][:, s * 384:(s + 1) * 384],
                                  Wqkv[0:128, s * 384:(s + 1) * 384])
            for s in range(2):
                nc.sync.dma_start(a_t[0][:, s * CQ:(s + 1) * CQ],
                                  A[0:128, s * CQ:(s + 1) * CQ])
            # block (kc, piece) = rows (kc*2+piece)*128..+128 of A is one
            # contiguous read; weights + piece0 interleaved so the kc-outer
            # QKV chains can start after the first pair lands.
            for kc in range(1, KC):
                nc.sync.dma_start(w_t[kc][:], Wqkv[kc * 128:(kc + 1) * 128, :])
                blk = kc * 2 * 128
                nc.sync.dma_start(a_t[kc][:, 0:T // 2],
                                  A[blk:blk + 128, :])
            for m in range(2):
                nc.sync.dma_start(bq_t[m][:], BQ[m * 128:(m + 1) * 128, :])
            nc.sync.dma_start(cst[:], CST[:, :])
            for kc in range(KC):
                blk = (kc * 2 + 1) * 128
                nc.sync.dma_start(a_t[kc][:, T // 2:T],
                                  A[blk:blk + 128, :])
            wp_t = []
            for kc in range(2):
                wp = pool.tile([128, C], DT, tag=f"WP{kc}", name=f"wp{kc}")
                nc.sync.dma_start(wp[:], WpT[kc * 128:(kc + 1) * 128, :])
                wp_t.append(wp)
            ident_t = cst[:, 0:128]    # identity
            tri_t = cst[:, 128:256]    # -1e9 strictly-lower triangle (r > j)

            ones_r = pool.tile([1, 64], f32, tag="ones1")
            nc.vector.memset(ones_r[:], 1.0)

            # persistent intermediates
            qt_sb = [pool.tile([128, T], DT, tag=f"QT{i}", name=f"qt{i}")
                     for i in range(2)]
            # per-head K aligned with where Q_h sits in the stacked QT tile
            # (rows hp:hp+64), other 64 rows zero - S matmuls then run
            # contraction 128 (contraction<=64 executes at half PE rate)
            # and the other head's Q rows hit the zeros
            kt_z = [pool.tile([128, T], DT, tag=f"KZ{h}", name=f"ktz{h}")
                    for h in range(H_LOC)]
            for h in range(H_LOC):
                zp = 0 if h % 2 else D
                nc.vector.memset(kt_z[h][zp:zp + D, :], 0.0)
            # V natural layout, all 16 token-tiles in one tile:
            # slice (tt, h) = [:, tt*4*VW + h*VW : +VW], col 64 = ones.
            v_all = pool.tile([128, NT * H_LOC * VW], DT, tag="VALL",
                              name="v_all")
            ao_sb = [pool.tile([128, T], DT, tag=f"AO{i}", name=f"ao{i}")
                     for i in range(2)]

            # ones columns of V: one strided memset covers all (tt, h)
            nc.vector.memset(
                v_all.rearrange("p (t x) -> p t x", x=VW)[:, :, D:D + 1], 1.0)

            # warm dve custom-op ucode path
            warm = wpool.tile([1, 8], f32, tag="warm")
            nc.vector.memset(warm[:], 1.0)
            warm2 = wpool.tile([1, 8], f32, tag="warm2")
            nc.vector.tensor_copy(warm2[:], warm[:])
            with nc.allow_low_precision(reason="warmup"):
                nc.vector.reciprocal_approx_fast(warm[:], warm2[:])
            if not USE_PE_MASK:
                warm3 = wpool.tile([128, 8], f32, tag="warm3")
                nc.vector.memset(warm3[:], 1.0)
                nc.gpsimd.affine_select(
                    out=warm3[:], in_=warm3[:],
                    compare_op=mybir.AluOpType.is_ge, fill=0.0, base=0,
                    pattern=[[1, 8]], channel_multiplier=-1)
            if not USE_F32R_BCAST:
                warm4 = wpool.tile([64, 8], f32, tag="warm4")
                nc.gpsimd.partition_broadcast(warm4[:], warm2[:])

            # ---------------- phase 1a: QKV halfA (q cols 0:1024) ----------
            # kc-outer over 8 chains: m in {q0,q1,k0,k1} x n in {0,1}.
            # slots: q0/q1 -> mm bufs, k0 -> pv bufs, k1 -> prj bufs
            mmA = psum.tile([128, 2 * CQ], f32, tag="mm", bufs=2, name="mmA")
            mmB = psum.tile([128, 2 * CQ], f32, tag="mm", bufs=2, name="mmB")
            pvA = psum.tile([128, CQ], f32, tag="pv", bufs=2, name="pvA")
            pvB = psum.tile([128, CQ], f32, tag="pv", bufs=2, name="pvB")
            prA = psum.tile([128, CQ], f32, tag="prj", bufs=2, name="prA")
            prB = psum.tile([128, CQ], f32, tag="prj", bufs=2, name="prB")
            halfA_slots = {
                (0, 0): mmA[:, 0:CQ], (0, 1): mmA[:, CQ:2 * CQ],
                (1, 0): mmB[:, 0:CQ], (1, 1): mmB[:, CQ:2 * CQ],
                (2, 0): pvA[:], (2, 1): pvB[:],
                (3, 0): prA[:], (3, 1): prB[:],
            }
            for kc in range(KC):
                for m in range(4):
                    for n in range(2):
                        nc.tensor.matmul(
                            halfA_slots[(m, n)],
                            w_t[kc][:, m * 128:(m + 1) * 128],
                            a_t[kc][:, n * CQ:(n + 1) * CQ],
                            start=(kc == 0), stop=(kc == KC - 1))
            # evict: Q via Scalar (fused bias add), K via DVE split per head
            for m in range(2):
                nc.scalar.activation(
                    qt_sb[m][:, 0:2 * CQ], (mmA if m == 0 else mmB)[:],
                    Ident, bias=bq_t[m][:, 0:1], scale=1.0)
            for ps_, h2, cols in [(pvA, 0, slice(0, CQ)),
                                  (pvB, 0, slice(CQ, 2 * CQ)),
                                  (prA, 2, slice(0, CQ)),
                                  (prB, 2, slice(CQ, 2 * CQ))]:
                nc.vector.tensor_copy(kt_z[h2][0:D, cols], ps_[0:D, :])
                nc.vector.tensor_copy(kt_z[h2 + 1][D:128, cols], ps_[D:128, :])

            # ---------------- attention stream -----------------------------
            # blocks (qj, h): chunks = diagonals (kc=qj*4..qj*4+3, trimmed to
            # the causal q-suffix) then off-diagonals (kc=0..qj*4-1).
            # Chunks pack pairwise into [128, <=1024] S tiles; one exp per
            # tile. PV accumulates per block in pv psum rows 0:65 (row 64 =
            # softmax denominator via the V ones column).
            class Chunk:
                __slots__ = ("kc", "qoff", "width", "off", "diag", "idx")

            class Tile:
                __slots__ = ("chunks", "ext", "blk", "first", "last", "pt")

            class Block:
                __slots__ = ("qj", "h", "tiles", "pv", "ls", "r", "rbs",
                             "n_chunks")

            blocks = []
            all_tiles = []
            for qj in range(4):
                for h in range(H_LOC):
                    blk = Block()
                    blk.qj, blk.h = qj, h
                    chunks = []
                    for kc in range(qj * 4):    # off-diagonal chunks first
                        c = Chunk()
                        c.kc, c.qoff, c.width, c.diag = kc, 0, CQ, False
                        chunks.append(c)
                    for i in range(4):          # diagonal chunks (trimmed)
                        c = Chunk()
                        c.kc = qj * 4 + i
                        c.qoff = i * 128
                        c.width = CQ - c.qoff
                        c.diag = True
                        chunks.append(c)
                    for ci, c in enumerate(chunks):
                        c.idx = ci
                    blk.n_chunks = len(chunks)
                    # pack pairs into [128,1024] tiles; each chunk owns one
                    # PSUM bank (accumulation groups must not share a bank),
                    # so the second chunk always starts at col 512
                    tiles = []
                    for j in range(0, len(chunks), 2):
                        t = Tile()
                        pair = chunks[j:j + 2]
                        pair[0].off = 0
                        if len(pair) == 2:
                            pair[1].off = CQ
                            t.ext = CQ + pair[1].width
                        else:
                            t.ext = pair[0].width
                        t.chunks = pair
                        t.blk = blk
                        tiles.append(t)
                    for ti, t in enumerate(tiles):
                        t.first = (ti == 0)
                        t.last = (ti == len(tiles) - 1)
                    blk.tiles = tiles
                    blocks.append(blk)
                    all_tiles.extend(tiles)

            n_tiles = len(all_tiles)

            def emit_S(t):
                blk = t.blk
                ht, hp = blk.h // 2, (blk.h % 2) * 64
                q0 = blk.qj * CQ
                sp = psum.tile([128, 2 * CQ], f32, tag="mm", bufs=2,
                               name="sp")
                for c in t.chunks:
                    nc.tensor.matmul(
                        sp[:, c.off:c.off + c.width],
                        kt_z[blk.h][:, c.kc * CK:(c.kc + 1) * CK],
                        qt_sb[ht][:, q0 + c.qoff:q0 + CQ],
                        start=True, stop=not (USE_PE_MASK and c.diag))
                    if USE_PE_MASK and c.diag:
                        nc.tensor.matmul(
                            sp[:, c.off:c.off + CK],
                            ident_t, tri_t,
                            start=False, stop=True, skip_group_check=True)
                pt = wpool.tile([128, 2 * CQ], DT, tag="pT", bufs=6)
                nc.scalar.activation(pt[:, 0:t.ext], sp[:, 0:t.ext],
                                     Exp, scale=1.0 / 8.0)
                if not USE_PE_MASK:
                    for c in t.chunks:
                        if c.diag:
                            w = min(CK, c.width)
                            nc.gpsimd.affine_select(
                                out=pt[:, c.off:c.off + w],
                                in_=pt[:, c.off:c.off + w],
                                compare_op=mybir.AluOpType.is_ge,
                                fill=0.0, base=0,
                                pattern=[[1, w]], channel_multiplier=-1)
                t.pt = pt

            def emit_PV(t):
                blk = t.blk
                if t.first:
                    blk.pv = psum.tile([128, CQ], f32, tag="pv", bufs=2,
                                       name="pv")
                for c in t.chunks:
                    nc.tensor.matmul(
                        blk.pv[0:VW, c.qoff:c.qoff + c.width],
                        v_all[:, c.kc * H_LOC * VW + blk.h * VW:
                              c.kc * H_LOC * VW + (blk.h + 1) * VW],
                        t.pt[:, c.off:c.off + c.width],
                        start=(c.idx == 0), stop=(c.idx == blk.n_chunks - 1))

            def emit_ls_recip(blk):
                ls = wpool.tile([1, CQ], f32, tag="ls", bufs=2)
                nc.vector.tensor_copy(ls[:], blk.pv[D:D + 1, :])
                r = wpool.tile([1, CQ], f32, tag="r", bufs=2)
                with nc.allow_low_precision(reason="softmax denom"):
                    nc.vector.reciprocal_approx_fast(r[:], ls[:])
                blk.ls, blk.r = ls, r

            def emit_bcast(blk):
                if USE_F32R_BCAST:
                    nc.tensor.matmul(
                        blk.pv[D:D + D, :],
                        ones_r[:].bitcast(f32r),
                        blk.r[:].bitcast(f32r),
                        start=True, stop=True, skip_group_check=True)
                else:
                    rbs = wpool.tile([D, CQ], f32, tag="rbs", bufs=2)
                    nc.gpsimd.partition_broadcast(rbs[:], blk.r[:])
                    blk.rbs = rbs

            def emit_mul(blk):
                ht, hp = blk.h // 2, (blk.h % 2) * 64
                q0 = blk.qj * CQ
                rhs = blk.pv[D:2 * D, :] if USE_F32R_BCAST else blk.rbs[:]
                nc.vector.tensor_mul(
                    ao_sb[ht][hp:hp + D, q0:q0 + CQ],
                    blk.pv[0:D, :], rhs)

            # background PE work: halfB QKV, V tt8-15, proj per qj
            background = []

            def bg_halfB(m, n):
                def run():
                    ps = psum.tile([128, CQ], f32, tag="prj", bufs=2,
                                   name="hb")
                    for kc in range(KC):
                        nc.tensor.matmul(
                            ps[:], w_t[kc][:, m * 128:(m + 1) * 128],
                            a_t[kc][:, n * CQ:(n + 1) * CQ],
                            start=(kc == 0), stop=(kc == KC - 1))
                    if m < 2:
                        nc.vector.tensor_scalar_add(
                            qt_sb[m][:, n * CQ:(n + 1) * CQ], ps[:],
                            bq_t[m][:, 0:1])
                    else:
                        h2 = (m - 2) * 2
                        cols = slice(n * CQ, (n + 1) * CQ)
                        nc.vector.tensor_copy(kt_z[h2][0:D, cols],
                                              ps[0:D, :])
                        nc.vector.tensor_copy(kt_z[h2 + 1][D:128, cols],
                                              ps[D:128, :])
                return run

            def bg_v(tt, tag="prj"):
                def run():
                    ps = psum.tile([128, CQ], f32, tag=tag, bufs=2,
                                   name="vb")
                    for kc in range(KC):
                        nc.tensor.matmul(
                            ps[:, 0:OL],
                            a_t[kc][:, tt * 128:(tt + 1) * 128],
                            w_t[kc][:, 2 * OL:3 * OL],
                            start=(kc == 0), stop=(kc == KC - 1))
                    dst = v_all.rearrange("p (t h x) -> p t h x",
                                          h=H_LOC, x=VW)[:, tt, :, 0:D]
                    src = ps[:, 0:OL].rearrange("p (h x) -> p h x", x=D)
                    nc.vector.tensor_copy(dst, src)
                return run

            def bg_proj(tt, n, last_qj):
                def run():
                    ps = psum.tile([128, CQ], f32, tag="prj", bufs=2,
                                   name="pj")
                    for kc2 in range(2):
                        nc.tensor.matmul(
                            ps[:],
                            ao_sb[kc2][:, tt * 128:(tt + 1) * 128],
                            wp_t[kc2][:, n * CQ:(n + 1) * CQ],
                            start=(kc2 == 0), stop=(kc2 == 1))
                    yt = wpool.tile([128, CQ], DT, tag="y", bufs=6)
                    if last_qj and n == 1:
                        # Scalar is idle after the final exp; split the tail
                        # evictions across engines
                        nc.scalar.copy(yt[:], ps[:])
                    else:
                        nc.vector.tensor_copy(yt[:], ps[:])
                    nc.sync.dma_start(
                        Y[tt * 128:(tt + 1) * 128, n * CQ:(n + 1) * CQ],
                        yt[:])
                return run

            # background order: V tt0-7 first (v0-3 popped before the loop,
            # the rest under qj0's exp cover), halfB n=2 (needed by qj2 S),
            # V tt8-11 (qj2 PV), halfB n=3 (qj3 S), V tt12-15 (qj3 PV);
            # proj chunks are appended as their qj completes.
            for tt in range(0, 8):
                background.append(bg_v(tt))
            for m in range(4):
                background.append(bg_halfB(m, 2))
            for tt in range(8, 12):
                background.append(bg_v(tt))
            for m in range(4):
                background.append(bg_halfB(m, 3))
            for tt in range(12, 16):
                background.append(bg_v(tt))

            # schedule: software pipeline over tiles with lookahead LA for
            # S/exp; block-end ops lag to keep engine queues stall-free.
            LA = 2
            pend_bcast = []   # (due_step, blk)
            pend_mul = []     # (due_step, blk)

            for i in range(min(LA, n_tiles)):
                emit_S(all_tiles[i])
            # V tt0-3 are needed by the first PVs; spread them across the
            # free pv+prj banks so their evictions don't serialize the PE
            background.pop(0)
            bg_v(0, tag="pv")()
            bg_v(1, tag="pv")()
            background.pop(0)
            for _ in range(2):
                background.pop(0)()
            quota = [4, 4, 8, 999]        # background pops allowed per qj

            for i, t in enumerate(all_tiles):
                for due, blk in [p for p in pend_bcast if p[0] <= i]:
                    emit_bcast(blk)
                    pend_bcast.remove((due, blk))
                if i + LA < n_tiles:
                    emit_S(all_tiles[i + LA])
                emit_PV(t)
                if t.last:
                    blk = t.blk
                    emit_ls_recip(blk)
                    pend_bcast.append((i + 1, blk))
                    pend_mul.append((i + 2, blk))
                for due, blk in [p for p in pend_mul if p[0] <= i]:
                    emit_mul(blk)
                    pend_mul.remove((due, blk))
                    if blk.h == H_LOC - 1:
                        qj = blk.qj
                        for tt in range(qj * 4, (qj + 1) * 4):
                            for n in range(2):
                                background.append(
                                    bg_proj(tt, n, qj == 3))
                # pop background work under the exp cover, budgeted per qj
                # so the in-order PE queue never starves Scalar of S tiles
                # (qj3 has the most Scalar slack, so proj work lands there).
                # Near the end, stretch the last few ops so the PE has fill
                # work while Scalar finishes the final exps.
                if background:
                    qj = t.blk.qj
                    backlog = len(background)
                    pop = quota[qj] > 0 or backlog >= n_tiles - i
                    if (qj == 3 and backlog <= 10 and i % 2 == 0
                            and backlog < n_tiles - i):
                        pop = False
                    if pop:
                        background.pop(0)()
                        quota[qj] -= 1

            # drain
            for _, blk in pend_bcast:
                emit_bcast(blk)
            for _, blk in pend_mul:
                emit_mul(blk)
                if blk.h == H_LOC - 1:
                    qj = blk.qj
                    for tt in range(qj * 4, (qj + 1) * 4):
                        for n in range(2):
                            background.append(bg_proj(tt, n, qj == 3))
            while background:
                background.pop(0)()

            if DEBUG_DUMP:
                for i in range(2):
                    nc.sync.dma_start(DQT[i * 128:(i + 1) * 128, :],
                                      qt_sb[i][:])
                    nc.sync.dma_start(DAO[i * 128:(i + 1) * 128, :],
                                      ao_sb[i][:])
                for h in range(H_LOC):
                    hp = (h % 2) * D
                    nc.sync.dma_start(DKT[h * D:(h + 1) * D, :],
                                      kt_z[h][hp:hp + D, :])
                nc.sync.dma_start(DV[:, :], v_all[:])

    nc.compile()
    return nc


def _get_compiled():
    global _COMPILED
    if _COMPILED is None:
        _COMPILED = _build()
    return _COMPILED


def _make_consts():
    ident = np.eye(128, dtype=np.float32)
    tri = np.zeros((128, 128), dtype=np.float32)
    r_idx = np.arange(128)[:, None]
    j_idx = np.arange(128)[None, :]
    tri[r_idx > j_idx] = -1e9
    return np.concatenate([ident, tri], axis=1).astype(NP_DT)


def make_in_maps(x, Wq, bq, Wk, Wv, Wp):
    cst = _make_consts()
    in_maps = []
    for c in range(N_CORES):
        b, g = divmod(c, 4)
        sl = slice(g * OL, (g + 1) * OL)
        in_maps.append({
            "A": np.ascontiguousarray(
                x[b].T.reshape(KC, 128, 2, T // 2).transpose(0, 2, 1, 3)
                .reshape(2 * C, T // 2)).astype(NP_DT),
            "Wqkv": np.concatenate(
                [Wq[sl].T, Wk[sl].T, Wv[sl].T], axis=1).astype(NP_DT),
            "WpT": np.ascontiguousarray(Wp[:, sl].T).astype(NP_DT),
            "BQ": bq[sl].reshape(OL, 1).astype(np.float32),
            "CST": cst,
        })
    return in_maps


_RUNNER = None


def _make_runner():
    """Build the 8-core shard_map executable once (run_bass_via_pjrt re-jits
    on every call; this caches the traced/compiled callable)."""
    import jax
    from jax.sharding import Mesh, PartitionSpec
    from jax.experimental.shard_map import shard_map
    import concourse.mybir as mybir_
    from concourse import bass2jax

    nc = _get_compiled()
    bass2jax.install_neuronx_cc_hook()

    partition_name = (nc.partition_id_tensor.name
                      if nc.partition_id_tensor else None)
    in_names, out_names, out_avals, zero_outs = [], [], [], []
    for alloc in nc.m.functions[0].allocations:
        if not isinstance(alloc, mybir_.MemoryLocationSet):
            continue
        name = alloc.memorylocations[0].name
        if alloc.kind == "ExternalInput":
            if name != partition_name:
                in_names.append(name)
        elif alloc.kind == "ExternalOutput":
            shape = tuple(alloc.tensor_shape)
            dtype = mybir_.dt.np(alloc.dtype)
            out_names.append(name)
            out_avals.append(jax.core.ShapedArray(shape, dtype))
            zero_outs.append(np.zeros(shape, dtype))
    n_params = len(in_names)
    n_outs = len(out_avals)
    all_in_names = list(in_names) + list(out_names)
    if partition_name is not None:
        all_in_names.append(partition_name)
    donate = tuple(range(n_params, n_params + n_outs))

    def _body(*args):
        operands = list(args)
        if partition_name is not None:
            operands.append(bass2jax.partition_id_tensor())
        outs = bass2jax._bass_exec_p.bind(
            *operands,
            out_avals=tuple(out_avals),
            in_names=tuple(all_in_names),
            out_names=tuple(out_names),
            lowering_input_output_aliases=(),
            sim_require_finite=True,
            sim_require_nnan=True,
            nc=nc,
        )
        return tuple(outs)

    devices = jax.devices()[:N_CORES]
    mesh = Mesh(np.asarray(devices), ("core",))
    in_specs = (PartitionSpec("core"),) * (n_params + n_outs)
    out_specs = (PartitionSpec("core"),) * n_outs
    sharded = jax.jit(
        shard_map(_body, mesh=mesh, in_specs=in_specs, out_specs=out_specs,
                  check_rep=False),
        donate_argnums=donate, keep_unused=True)

    def run(in_maps):
        per_core = [[np.asarray(m[name]) for name in in_names]
                    for m in in_maps]
        concat_in = [
            np.concatenate([per_core[c][i] for c in range(N_CORES)], axis=0)
            for i in range(n_params)]
        concat_zeros = [
            np.zeros((N_CORES * z.shape[0], *z.shape[1:]), z.dtype)
            for z in zero_outs]
        out_arrs = sharded(*concat_in, *concat_zeros)
        return [
            {name: np.asarray(out_arrs[i]).reshape(
                N_CORES, *out_avals[i].shape)[c]
             for i, name in enumerate(out_names)}
            for c in range(N_CORES)]

    return run


def _get_runner():
    global _RUNNER
    if _RUNNER is None:
        _RUNNER = _make_runner()
    return _RUNNER


def _axon_reset():
    try:
        import ctypes
        lib = ctypes.CDLL("/opt/axon/libaxon_pjrt.so")
        if hasattr(lib, "axon_reset"):
            lib.axon_reset()
    except Exception:
        pass


def kernel(x, Wq, bq, Wk, bk, Wv, bv, Wp, bp):
    x = np.asarray(x, dtype=np.float32)
    Wq = np.asarray(Wq, dtype=np.float32)
    bq = np.asarray(bq, dtype=np.float32)
    Wk = np.asarray(Wk, dtype=np.float32)
    Wv = np.asarray(Wv, dtype=np.float32)
    Wp = np.asarray(Wp, dtype=np.float32)
    bv = np.asarray(bv, dtype=np.float32)
    bp = np.asarray(bp, dtype=np.float32)

    in_maps = make_in_maps(x, Wq, bq, Wk, Wv, Wp)

    results = None
    for attempt in range(3):
        try:
            results = _get_runner()(in_maps)
            break
        except Exception:
            if attempt == 2:
                raise
            _axon_reset()  # recover a wedged accelerator and retry

    extra = bv @ Wp.T + bp  # bv/bp fold out of the device kernel
    out = np.empty((B, T, C), dtype=np.float32)
    for b in range(B):
        acc = results[4 * b]["Y"].astype(np.float32)
        for g in range(1, 4):
            acc = acc + results[4 * b + g]["Y"].astype(np.float32)
        out[b] = acc + extra
    return out


# revision 57
# speedup vs baseline: 1.3144x; 1.0182x over previous
"""Multi-head causal attention (B=2, T=2048, C=1024, H=16, D=64) on 8 trn2 cores.

Sharding: core c -> batch b = c//4, head group g = c%4 (4 heads each),
Megatron-style: QKV column-parallel, proj row-parallel. Partial outputs are
summed on the host; bk is softmax-invariant and dropped, bv/bp fold into a
host-side constant. All matmul operands are bf16 (host-cast), accumulation
and softmax stay fp32.

v2: the attention window is paced by the Scalar engine (exp is Scalar-only at
~0.83ns/row), so the kernel keeps Scalar 100% on Exp and moves everything
else off it:
  - exp runs on [128, <=1024] paired S tiles (2 PSUM banks) - 80 calls, no
    wasted rows (diag chunks pack densely).
  - causal mask = PE matmul accumulating a -1e9 triangle constant into S
    BEFORE exp (no GpSimd affine_select, no post-exp dependency).
  - softmax denominators: ones-column in V gives l in PV row 64; GpSimd
    copies it out, DVE approx-reciprocal, PE broadcasts r back into PV rows
    64:128 via an f32r rank-1 matmul, one DVE multiply -> ao.
  - QKV phase 1 runs kc-outer over 8 concurrent PSUM chains so the PE starts
    ~1us in (first weight chunk) instead of waiting for the full load; only
    the first half (q cols 0:1024) runs up front - the second half, V[tt>=8]
    and the per-qj projection are interleaved as background PE work under
    the Scalar-bound attention window.
  - Y is stored bf16 (host sums partials in f32).
"""

import sys

sys.path.insert(0, "/opt/trn_rl_repo")

import numpy as np
import ml_dtypes

NP_DT = ml_dtypes.bfloat16

import concourse.bass as bass  # noqa: F401
import concourse.mybir as mybir
import concourse.tile as tile
from concourse import bacc
from concourse.bass_utils import run_bass_kernel_spmd  # noqa: F401

N_CORES = 8
B, T, C = 2, 2048, 1024
H, D = 16, 64
H_LOC = 4              # heads per core
OL = H_LOC * D         # local channels = 256
CQ = 512               # q chunk (PSUM bank / block granularity)
CK = 128               # k chunk (partition dim)
NT = T // 128          # 16
KC = C // 128          # 8 contraction chunks for QKV
VW = D + 1             # 65: V columns per head incl ones column

f32 = mybir.dt.float32
f32r = mybir.dt.float32r
bf16 = mybir.dt.bfloat16
DT = bf16

USE_PE_MASK = False    # mask via PE matmul add of -1e9 triangle (else gpsimd)
USE_F32R_BCAST = False  # broadcast 1/l via f32r PE matmul (else gpsimd)
DEBUG_DUMP = False     # add QT/KT/V/AO dram dumps for debugging

_COMPILED = None


def _build():
    nc = bacc.Bacc("TRN2", debug=False, num_devices=N_CORES)

    A = nc.dram_tensor("A", [2 * C, T // 2], DT, kind="ExternalInput").ap()
    Wqkv = nc.dram_tensor("Wqkv", [C, 3 * OL], DT, kind="ExternalInput").ap()
    WpT = nc.dram_tensor("WpT", [OL, C], DT, kind="ExternalInput").ap()
    BQ = nc.dram_tensor("BQ", [OL, 1], f32, kind="ExternalInput").ap()
    CST = nc.dram_tensor("CST", [128, 256], DT, kind="ExternalInput").ap()
    Y = nc.dram_tensor("Y", [T, C], DT, kind="ExternalOutput").ap()
    if DEBUG_DUMP:
        DQT = nc.dram_tensor("DQT", [256, T], DT, kind="ExternalOutput").ap()
        DKT = nc.dram_tensor("DKT", [256, T], DT, kind="ExternalOutput").ap()
        DV = nc.dram_tensor("DV", [128, NT * H_LOC * VW], DT,
                            kind="ExternalOutput").ap()
        DAO = nc.dram_tensor("DAO", [256, T], DT, kind="ExternalOutput").ap()

    Exp = mybir.ActivationFunctionType.Exp
    Ident = mybir.ActivationFunctionType.Identity

    with tile.TileContext(nc) as tc:
        with tc.tile_pool(name="sbuf", bufs=1) as pool, \
             tc.tile_pool(name="work", bufs=1) as wpool, \
             tc.tile_pool(name="psum", bufs=1, space="PSUM") as psum:

            # ---------------- resident inputs ----------------
            a_t = [pool.tile([128, T], DT, tag=f"A{kc}", name=f"a{kc}")
                   for kc in range(KC)]
            w_t = [pool.tile([128, 3 * OL], DT, tag=f"W{kc}", name=f"w{kc}")
                   for kc in range(KC)]
            # kc=0 pair first, in strips (parallel DMA engines finish each
            # strip sooner), so the first QKV chain starts as early as
            # possible; bq/cst are tiny and only needed ~10us in.
            bq_t = [pool.tile([128, 1], f32, tag=f"BQ{m}", name=f"bq{m}")
                    for m in range(2)]
            cst = pool.tile([128, 256], DT, tag="CST", name="cst")
            # issue the first kc pair from FOUR queues in parallel - each
            # dma_start costs ~600ns of issuing-queue time, and sync alone
            # would serialize them after the preamble
            nc.sync.dma_start(w_t[0][:, 0:384], Wqkv[0:128, 0:384])
            nc.scalar.dma_start(a_t[0][:, 0:CQ], A[0:128, 0:CQ])
            nc.gpsimd.dma_start(w_t[0][:, 384:768], Wqkv[0:128, 384:768])
            nc.sync.dma_start(a_t[0][:, CQ:2 * CQ], A[0:128, CQ:2 * CQ])
            # block (kc, piece) = rows (kc*2+piece)*128..+128 of A is one
            # contiguous read; weights + piece0 interleaved so the kc-outer
            # QKV chains can start after the first pair lands. kc 1-3 loads
            # ride the still-idle vector/scalar queues.
            for kc in range(1, KC):
                if kc < 4:
                    nc.scalar.dma_start(w_t[kc][:],
                                        Wqkv[kc * 128:(kc + 1) * 128, :])
                    blk = kc * 2 * 128
                    nc.gpsimd.dma_start(a_t[kc][:, 0:T // 2],
                                        A[blk:blk + 128, :])
                else:
                    nc.sync.dma_start(w_t[kc][:],
                                      Wqkv[kc * 128:(kc + 1) * 128, :])
                    blk = kc * 2 * 128
                    nc.sync.dma_start(a_t[kc][:, 0:T // 2],
                                      A[blk:blk + 128, :])
            for m in range(2):
                nc.gpsimd.dma_start(bq_t[m][:], BQ[m * 128:(m + 1) * 128, :])
            nc.gpsimd.dma_start(cst[:], CST[:, :])
            for kc in range(KC):
                blk = (kc * 2 + 1) * 128
                nc.sync.dma_start(a_t[kc][:, T // 2:T],
                                  A[blk:blk + 128, :])
            wp_t = []
            for kc in range(2):
                wp = pool.tile([128, C], DT, tag=f"WP{kc}", name=f"wp{kc}")
                nc.sync.dma_start(wp[:], WpT[kc * 128:(kc + 1) * 128, :])
                wp_t.append(wp)
            ident_t = cst[:, 0:128]    # identity
            tri_t = cst[:, 128:256]    # -1e9 strictly-lower triangle (r > j)

            ones_r = pool.tile([1, 64], f32, tag="ones1")
            nc.vector.memset(ones_r[:], 1.0)

            # persistent intermediates
            qt_sb = [pool.tile([128, T], DT, tag=f"QT{i}", name=f"qt{i}")
                     for i in range(2)]
            # per-head K aligned with where Q_h sits in the stacked QT tile
            # (rows hp:hp+64), other 64 rows zero - S matmuls then run
            # contraction 128 (contraction<=64 executes at half PE rate)
            # and the other head's Q rows hit the zeros
            kt_z = [pool.tile([128, T], DT, tag=f"KZ{h}", name=f"ktz{h}")
                    for h in range(H_LOC)]
            for h in range(H_LOC):
                zp = 0 if h % 2 else D
                nc.vector.memset(kt_z[h][zp:zp + D, :], 0.0)
            # V natural layout, all 16 token-tiles in one tile:
            # slice (tt, h) = [:, tt*4*VW + h*VW : +VW], col 64 = ones.
            v_all = pool.tile([128, NT * H_LOC * VW], DT, tag="VALL",
                              name="v_all")
            ao_sb = [pool.tile([128, T], DT, tag=f"AO{i}", name=f"ao{i}")
                     for i in range(2)]

            # ones columns of V: one strided memset covers all (tt, h)
            nc.vector.memset(
                v_all.rearrange("p (t x) -> p t x", x=VW)[:, :, D:D + 1], 1.0)

            # warm dve custom-op ucode path
            warm = wpool.tile([1, 8], f32, tag="warm")
            nc.vector.memset(warm[:], 1.0)
            warm2 = wpool.tile([1, 8], f32, tag="warm2")
            nc.vector.tensor_copy(warm2[:], warm[:])
            with nc.allow_low_precision(reason="warmup"):
                nc.vector.reciprocal_approx_fast(warm[:], warm2[:])
            if not USE_PE_MASK:
                warm3 = wpool.tile([128, 8], f32, tag="warm3")
                nc.vector.memset(warm3[:], 1.0)
                nc.gpsimd.affine_select(
                    out=warm3[:], in_=warm3[:],
                    compare_op=mybir.AluOpType.is_ge, fill=0.0, base=0,
                    pattern=[[1, 8]], channel_multiplier=-1)
            if not USE_F32R_BCAST:
                warm4 = wpool.tile([64, 8], f32, tag="warm4")
                nc.gpsimd.partition_broadcast(warm4[:], warm2[:])

            # ---------------- phase 1a: QKV halfA (q cols 0:1024) ----------
            # kc-outer over 8 chains: m in {q0,q1,k0,k1} x n in {0,1}.
            # slots: q0/q1 -> mm bufs, k0 -> pv bufs, k1 -> prj bufs
            mmA = psum.tile([128, 2 * CQ], f32, tag="mm", bufs=2, name="mmA")
            mmB = psum.tile([128, 2 * CQ], f32, tag="mm", bufs=2, name="mmB")
            pvA = psum.tile([128, CQ], f32, tag="pv", bufs=2, name="pvA")
            pvB = psum.tile([128, CQ], f32, tag="pv", bufs=2, name="pvB")
            prA = psum.tile([128, CQ], f32, tag="prj", bufs=2, name="prA")
            prB = psum.tile([128, CQ], f32, tag="prj", bufs=2, name="prB")
            halfA_slots = {
                (0, 0): mmA[:, 0:CQ], (0, 1): mmA[:, CQ:2 * CQ],
                (1, 0): mmB[:, 0:CQ], (1, 1): mmB[:, CQ:2 * CQ],
                (2, 0): pvA[:], (2, 1): pvB[:],
                (3, 0): prA[:], (3, 1): prB[:],
            }
            for kc in range(KC):
                for m in range(4):
                    for n in range(2):
                        nc.tensor.matmul(
                            halfA_slots[(m, n)],
                            w_t[kc][:, m * 128:(m + 1) * 128],
                            a_t[kc][:, n * CQ:(n + 1) * CQ],
                            start=(kc == 0), stop=(kc == KC - 1))
            # evict: Q via Scalar (fused bias add), K via DVE split per head
            for m in range(2):
                nc.scalar.activation(
                    qt_sb[m][:, 0:2 * CQ], (mmA if m == 0 else mmB)[:],
                    Ident, bias=bq_t[m][:, 0:1], scale=1.0)
            for ps_, h2, cols in [(pvA, 0, slice(0, CQ)),
                                  (pvB, 0, slice(CQ, 2 * CQ)),
                                  (prA, 2, slice(0, CQ)),
                                  (prB, 2, slice(CQ, 2 * CQ))]:
                nc.vector.tensor_copy(kt_z[h2][0:D, cols], ps_[0:D, :])
                nc.vector.tensor_copy(kt_z[h2 + 1][D:128, cols], ps_[D:128, :])

            # ---------------- attention stream -----------------------------
            # blocks (qj, h): chunks = diagonals (kc=qj*4..qj*4+3, trimmed to
            # the causal q-suffix) then off-diagonals (kc=0..qj*4-1).
            # Chunks pack pairwise into [128, <=1024] S tiles; one exp per
            # tile. PV accumulates per block in pv psum rows 0:65 (row 64 =
            # softmax denominator via the V ones column).
            class Chunk:
                __slots__ = ("kc", "qoff", "width", "off", "diag", "idx")

            class Tile:
                __slots__ = ("chunks", "ext", "blk", "first", "last", "pt")

            class Block:
                __slots__ = ("qj", "h", "tiles", "pv", "ls", "r", "rbs",
                             "n_chunks")

            blocks = []
            all_tiles = []
            for qj in range(4):
                for h in range(H_LOC):
                    blk = Block()
                    blk.qj, blk.h = qj, h
                    chunks = []
                    for kc in range(qj * 4):    # off-diagonal chunks first
                        c = Chunk()
                        c.kc, c.qoff, c.width, c.diag = kc, 0, CQ, False
                        chunks.append(c)
                    for i in range(4):          # diagonal chunks (trimmed)
                        c = Chunk()
                        c.kc = qj * 4 + i
                        c.qoff = i * 128
                        c.width = CQ - c.qoff
                        c.diag = True
                        chunks.append(c)
                    for ci, c in enumerate(chunks):
                        c.idx = ci
                    blk.n_chunks = len(chunks)
                    # pack pairs into [128,1024] tiles; each chunk owns one
                    # PSUM bank (accumulation groups must not share a bank),
                    # so the second chunk always starts at col 512
                    tiles = []
                    for j in range(0, len(chunks), 2):
                        t = Tile()
                        pair = chunks[j:j + 2]
                        pair[0].off = 0
                        if len(pair) == 2:
                            pair[1].off = CQ
                            t.ext = CQ + pair[1].width
                        else:
                            t.ext = pair[0].width
                        t.chunks = pair
                        t.blk = blk
                        tiles.append(t)
                    for ti, t in enumerate(tiles):
                        t.first = (ti == 0)
                        t.last = (ti == len(tiles) - 1)
                    blk.tiles = tiles
                    blocks.append(blk)
                    all_tiles.extend(tiles)

            n_tiles = len(all_tiles)

            def emit_S(t):
                blk = t.blk
                ht, hp = blk.h // 2, (blk.h % 2) * 64
                q0 = blk.qj * CQ
                sp = psum.tile([128, 2 * CQ], f32, tag="mm", bufs=2,
                               name="sp")
                for c in t.chunks:
                    nc.tensor.matmul(
                        sp[:, c.off:c.off + c.width],
                        kt_z[blk.h][:, c.kc * CK:(c.kc + 1) * CK],
                        qt_sb[ht][:, q0 + c.qoff:q0 + CQ],
                        start=True, stop=not (USE_PE_MASK and c.diag))
                    if USE_PE_MASK and c.diag:
                        nc.tensor.matmul(
                            sp[:, c.off:c.off + CK],
                            ident_t, tri_t,
                            start=False, stop=True, skip_group_check=True)
                pt = wpool.tile([128, 2 * CQ], DT, tag="pT", bufs=6)
                nc.scalar.activation(pt[:, 0:t.ext], sp[:, 0:t.ext],
                                     Exp, scale=1.0 / 8.0)
                if not USE_PE_MASK:
                    for c in t.chunks:
                        if c.diag:
                            w = min(CK, c.width)
                            nc.gpsimd.affine_select(
                                out=pt[:, c.off:c.off + w],
                                in_=pt[:, c.off:c.off + w],
                                compare_op=mybir.AluOpType.is_ge,
                                fill=0.0, base=0,
                                pattern=[[1, w]], channel_multiplier=-1)
                t.pt = pt

            def emit_PV(t):
                blk = t.blk
                if t.first:
                    blk.pv = psum.tile([128, CQ], f32, tag="pv", bufs=2,
                                       name="pv")
                for c in t.chunks:
                    nc.tensor.matmul(
                        blk.pv[0:VW, c.qoff:c.qoff + c.width],
                        v_all[:, c.kc * H_LOC * VW + blk.h * VW:
                              c.kc * H_LOC * VW + (blk.h + 1) * VW],
                        t.pt[:, c.off:c.off + c.width],
                        start=(c.idx == 0), stop=(c.idx == blk.n_chunks - 1))

            def emit_ls_recip(blk):
                ls = wpool.tile([1, CQ], f32, tag="ls", bufs=2)
                nc.vector.tensor_copy(ls[:], blk.pv[D:D + 1, :])
                r = wpool.tile([1, CQ], f32, tag="r", bufs=2)
                with nc.allow_low_precision(reason="softmax denom"):
                    nc.vector.reciprocal_approx_fast(r[:], ls[:])
                blk.ls, blk.r = ls, r

            def emit_bcast(blk):
                if USE_F32R_BCAST:
                    nc.tensor.matmul(
                        blk.pv[D:D + D, :],
                        ones_r[:].bitcast(f32r),
                        blk.r[:].bitcast(f32r),
                        start=True, stop=True, skip_group_check=True)
                else:
                    rbs = wpool.tile([D, CQ], f32, tag="rbs", bufs=2)
                    nc.gpsimd.partition_broadcast(rbs[:], blk.r[:])
                    blk.rbs = rbs

            def emit_mul(blk):
                ht, hp = blk.h // 2, (blk.h % 2) * 64
                q0 = blk.qj * CQ
                rhs = blk.pv[D:2 * D, :] if USE_F32R_BCAST else blk.rbs[:]
                nc.vector.tensor_mul(
                    ao_sb[ht][hp:hp + D, q0:q0 + CQ],
                    blk.pv[0:D, :], rhs)

            # background PE work: halfB QKV, V tt8-15, proj per qj
            background = []

            def bg_halfB(m, n):
                def run():
                    ps = psum.tile([128, CQ], f32, tag="prj", bufs=2,
                                   name="hb")
                    for kc in range(KC):
                        nc.tensor.matmul(
                            ps[:], w_t[kc][:, m * 128:(m + 1) * 128],
                            a_t[kc][:, n * CQ:(n + 1) * CQ],
                            start=(kc == 0), stop=(kc == KC - 1))
                    if m < 2:
                        nc.vector.tensor_scalar_add(
                            qt_sb[m][:, n * CQ:(n + 1) * CQ], ps[:],
                            bq_t[m][:, 0:1])
                    else:
                        h2 = (m - 2) * 2
                        cols = slice(n * CQ, (n + 1) * CQ)
                        nc.vector.tensor_copy(kt_z[h2][0:D, cols],
                                              ps[0:D, :])
                        nc.vector.tensor_copy(kt_z[h2 + 1][D:128, cols],
                                              ps[D:128, :])
                return run

            def bg_v(tt, tag="prj"):
                def run():
                    ps = psum.tile([128, CQ], f32, tag=tag, bufs=2,
                                   name="vb")
                    for kc in range(KC):
                        nc.tensor.matmul(
                            ps[:, 0:OL],
                            a_t[kc][:, tt * 128:(tt + 1) * 128],
                            w_t[kc][:, 2 * OL:3 * OL],
                            start=(kc == 0), stop=(kc == KC - 1))
                    dst = v_all.rearrange("p (t h x) -> p t h x",
                                          h=H_LOC, x=VW)[:, tt, :, 0:D]
                    src = ps[:, 0:OL].rearrange("p (h x) -> p h x", x=D)
                    nc.vector.tensor_copy(dst, src)
                return run

            def bg_proj(tt, n, last_qj):
                def run():
                    ps = psum.tile([128, CQ], f32, tag="prj", bufs=2,
                                   name="pj")
                    for kc2 in range(2):
                        nc.tensor.matmul(
                            ps[:],
                            ao_sb[kc2][:, tt * 128:(tt + 1) * 128],
                            wp_t[kc2][:, n * CQ:(n + 1) * CQ],
                            start=(kc2 == 0), stop=(kc2 == 1))
                    yt = wpool.tile([128, CQ], DT, tag="y", bufs=6)
                    if last_qj and n == 1:
                        # Scalar is idle after the final exp; split the tail
                        # evictions across engines
                        nc.scalar.copy(yt[:], ps[:])
                    else:
                        nc.vector.tensor_copy(yt[:], ps[:])
                    nc.sync.dma_start(
                        Y[tt * 128:(tt + 1) * 128, n * CQ:(n + 1) * CQ],
                        yt[:])
                return run

            # background order: V tt0-7 first (v0-3 popped before the loop,
            # the rest under qj0's exp cover), halfB n=2 (needed by qj2 S),
            # V tt8-11 (qj2 PV), halfB n=3 (qj3 S), V tt12-15 (qj3 PV);
            # proj chunks are appended as their qj completes.
            for tt in range(0, 8):
                background.append(bg_v(tt))
            for m in range(4):
                background.append(bg_halfB(m, 2))
            for tt in range(8, 12):
                background.append(bg_v(tt))
            for m in range(4):
                background.append(bg_halfB(m, 3))
            for tt in range(12, 16):
                background.append(bg_v(tt))

            # schedule: software pipeline over tiles with lookahead LA for
            # S/exp; block-end ops lag to keep engine queues stall-free.
            LA = 2
            pend_bcast = []   # (due_step, blk)
            pend_mul = []     # (due_step, blk)

            for i in range(min(LA, n_tiles)):
                emit_S(all_tiles[i])
            # V tt0-3 are needed by the first PVs; spread them across the
            # free pv+prj banks so their evictions don't serialize the PE
            background.pop(0)
            bg_v(0, tag="pv")()
            bg_v(1, tag="pv")()
            background.pop(0)
            for _ in range(2):
                background.pop(0)()
            quota = [4, 4, 8, 999]        # background pops allowed per qj

            for i, t in enumerate(all_tiles):
                for due, blk in [p for p in pend_bcast if p[0] <= i]:
                    emit_bcast(blk)
                    pend_bcast.remove((due, blk))
                if i + LA < n_tiles:
                    emit_S(all_tiles[i + LA])
                emit_PV(t)
                if t.last:
                    blk = t.blk
                    emit_ls_recip(blk)
                    pend_bcast.append((i + 1, blk))
                    pend_mul.append((i + 2, blk))
                for due, blk in [p for p in pend_mul if p[0] <= i]:
                    emit_mul(blk)
                    pend_mul.remove((due, blk))
                    if blk.h == H_LOC - 1:
                        qj = blk.qj
                        for tt in range(qj * 4, (qj + 1) * 4):
                            for n in range(2):
                                background.append(
                                    bg_proj(tt, n, qj == 3))
                # pop background work under the exp cover, budgeted per qj
                # so the in-order PE queue never starves Scalar of S tiles
                # (qj3 has the most Scalar slack, so proj work lands there).
                # Near the end, stretch the last few ops so the PE has fill
                # work while Scalar finishes the final exps.
                if background:
                    qj = t.blk.qj
                    backlog = len(background)
                    pop = quota[qj] > 0 or backlog >= n_tiles - i
                    if (qj == 3 and backlog <= 10 and i % 2 == 0
                            and backlog < n_tiles - i):
                        pop = False
                    if pop:
                        background.pop(0)()
                        quota[qj] -= 1

            # drain
            for _, blk in pend_bcast:
                emit_bcast(blk)
            for _, blk in pend_mul:
                emit_mul(blk)
                if blk.h == H_LOC - 1:
                    qj = blk.qj
                    for tt in range(qj * 4, (qj + 1) * 4):
                        for n in range(2):
                            background.append(bg_proj(tt, n, qj == 3))
            while background:
                background.pop(0)()

            if DEBUG_DUMP:
                for i in range(2):
                    nc.sync.dma_start(DQT[i * 128:(i + 1) * 128, :],
                                      qt_sb[i][:])
                    nc.sync.dma_start(DAO[i * 128:(i + 1) * 128, :],
                                      ao_sb[i][:])
                for h in range(H_LOC):
                    hp = (h % 2) * D
                    nc.sync.dma_start(DKT[h * D:(h + 1) * D, :],
                                      kt_z[h][hp:hp + D, :])
                nc.sync.dma_start(DV[:, :], v_all[:])

    nc.compile()
    return nc


def _get_compiled():
    global _COMPILED
    if _COMPILED is None:
        _COMPILED = _build()
    return _COMPILED


def _make_consts():
    ident = np.eye(128, dtype=np.float32)
    tri = np.zeros((128, 128), dtype=np.float32)
    r_idx = np.arange(128)[:, None]
    j_idx = np.arange(128)[None, :]
    tri[r_idx > j_idx] = -1e9
    return np.concatenate([ident, tri], axis=1).astype(NP_DT)


def make_in_maps(x, Wq, bq, Wk, Wv, Wp):
    cst = _make_consts()
    in_maps = []
    for c in range(N_CORES):
        b, g = divmod(c, 4)
        sl = slice(g * OL, (g + 1) * OL)
        in_maps.append({
            "A": np.ascontiguousarray(
                x[b].T.reshape(KC, 128, 2, T // 2).transpose(0, 2, 1, 3)
                .reshape(2 * C, T // 2)).astype(NP_DT),
            "Wqkv": np.concatenate(
                [Wq[sl].T, Wk[sl].T, Wv[sl].T], axis=1).astype(NP_DT),
            "WpT": np.ascontiguousarray(Wp[:, sl].T).astype(NP_DT),
            "BQ": bq[sl].reshape(OL, 1).astype(np.float32),
            "CST": cst,
        })
    return in_maps


_RUNNER = None


def _make_runner():
    """Build the 8-core shard_map executable once (run_bass_via_pjrt re-jits
    on every call; this caches the traced/compiled callable)."""
    import jax
    from jax.sharding import Mesh, PartitionSpec
    from jax.experimental.shard_map import shard_map
    import concourse.mybir as mybir_
    from concourse import bass2jax

    nc = _get_compiled()
    bass2jax.install_neuronx_cc_hook()

    partition_name = (nc.partition_id_tensor.name
                      if nc.partition_id_tensor else None)
    in_names, out_names, out_avals, zero_outs = [], [], [], []
    for alloc in nc.m.functions[0].allocations:
        if not isinstance(alloc, mybir_.MemoryLocationSet):
            continue
        name = alloc.memorylocations[0].name
        if alloc.kind == "ExternalInput":
            if name != partition_name:
                in_names.append(name)
        elif alloc.kind == "ExternalOutput":
            shape = tuple(alloc.tensor_shape)
            dtype = mybir_.dt.np(alloc.dtype)
            out_names.append(name)
            out_avals.append(jax.core.ShapedArray(shape, dtype))
            zero_outs.append(np.zeros(shape, dtype))
    n_params = len(in_names)
    n_outs = len(out_avals)
    all_in_names = list(in_names) + list(out_names)
    if partition_name is not None:
        all_in_names.append(partition_name)
    donate = tuple(range(n_params, n_params + n_outs))

    def _body(*args):
        operands = list(args)
        if partition_name is not None:
            operands.append(bass2jax.partition_id_tensor())
        outs = bass2jax._bass_exec_p.bind(
            *operands,
            out_avals=tuple(out_avals),
            in_names=tuple(all_in_names),
            out_names=tuple(out_names),
            lowering_input_output_aliases=(),
            sim_require_finite=True,
            sim_require_nnan=True,
            nc=nc,
        )
        return tuple(outs)

    devices = jax.devices()[:N_CORES]
    mesh = Mesh(np.asarray(devices), ("core",))
    in_specs = (PartitionSpec("core"),) * (n_params + n_outs)
    out_specs = (PartitionSpec("core"),) * n_outs
    sharded = jax.jit(
        shard_map(_body, mesh=mesh, in_specs=in_specs, out_specs=out_specs,
                  check_rep=False),
        donate_argnums=donate, keep_unused=True)

    def run(in_maps):
        per_core = [[np.asarray(m[name]) for name in in_names]
                    for m in in_maps]
        concat_in = [
            np.concatenate([per_core[c][i] for c in range(N_CORES)], axis=0)
            for i in range(n_params)]
        concat_zeros = [
            np.zeros((N_CORES * z.shape[0], *z.shape[1:]), z.dtype)
            for z in zero_outs]
        out_arrs = sharded(*concat_in, *concat_zeros)
        return [
            {name: np.asarray(out_arrs[i]).reshape(
                N_CORES, *out_avals[i].shape)[c]
             for i, name in enumerate(out_names)}
            for c in range(N_CORES)]

    return run


def _get_runner():
    global _RUNNER
    if _RUNNER is None:
        _RUNNER = _make_runner()
    return _RUNNER


def _axon_reset():
    try:
        import ctypes
        lib = ctypes.CDLL("/opt/axon/libaxon_pjrt.so")
        if hasattr(lib, "axon_reset"):
            lib.axon_reset()
    except Exception:
        pass


def kernel(x, Wq, bq, Wk, bk, Wv, bv, Wp, bp):
    x = np.asarray(x, dtype=np.float32)
    Wq = np.asarray(Wq, dtype=np.float32)
    bq = np.asarray(bq, dtype=np.float32)
    Wk = np.asarray(Wk, dtype=np.float32)
    Wv = np.asarray(Wv, dtype=np.float32)
    Wp = np.asarray(Wp, dtype=np.float32)
    bv = np.asarray(bv, dtype=np.float32)
    bp = np.asarray(bp, dtype=np.float32)

    in_maps = make_in_maps(x, Wq, bq, Wk, Wv, Wp)

    results = None
    for attempt in range(3):
        try:
            results = _get_runner()(in_maps)
            break
        except Exception:
            if attempt == 2:
                raise
            _axon_reset()  # recover a wedged accelerator and retry

    extra = bv @ Wp.T + bp  # bv/bp fold out of the device kernel
    out = np.empty((B, T, C), dtype=np.float32)
    for b in range(B):
        acc = results[4 * b]["Y"].astype(np.float32)
        for g in range(1, 4):
            acc = acc + results[4 * b + g]["Y"].astype(np.float32)
        out[b] = acc + extra
    return out
